# revision 60
# baseline (speedup 1.0000x reference)
"""CBOW forward (embedding lookup + pooled dot + weighted BCE) on 8 TRN2 cores.

Strategy: data-parallel over the batch; each core owns BL = 2048 examples.

The original kernel was bound by SWDGE descriptor generation: every gathered
row costs ~2.4 ns of serial GPSIMD (Pool engine) ucode time, and 36864
rows/core -> ~89 us.  This version removes ~80% of that by exploiting a
layout freedom: within an example, context positions (and negative slots,
together with their weight/label) are interchangeable.  Host-side we order
each example's rows so that rows which can be served by a *statically
shaped* dense stream come first:

  - For position/slot index j, a static "staircase" m_j says examples
    0..m_j-1 get row j from a dense stream tensor (packed host-side in
    exactly slot order), and examples >= m_j get it via dma_gather.
  - Each example's single-occurrence rows are placed in the dense slots
    first, so the dense stream is (nearly) duplication-free: it is a
    permutation of unique table rows, plus a few % duplicates where an
    example has fewer single-occurrence rows than dense slots.

Dense streams ride plain static DMAs on the two HWDGE queues (SP/ACT
engines), packed p-major host-side so each descriptor covers a 16-row
4 KB span (HWDGE charges ~1 ns/descriptor to the issuing engine).  Only
~6.5k rows/core (the staircase tails) still use dma_gather.

Context pooling runs on the otherwise-idle TensorEngine: identity-weight
matmuls accumulate the 10 position tiles into PSUM (f32), freeing the DVE
for the negative phase (mul + fold + reduce) and epilogue.

Tables are bf16; reduction and epilogue stay f32.
Host: per_row = num / sum_k(weight_mask); answer = mean over all rows.
"""

import numpy as np
import ml_dtypes

# run_bass_kernel_spmd under axon imports antenv.axon_hooks unconditionally;
# provide an in-process stub if the container image lacks that module.
import sys as _sys
import types as _types

try:
    import antenv.axon_hooks  # noqa: F401
except Exception:
    import antenv as _antenv

    _m = _types.ModuleType("antenv.axon_hooks")
    _m._hook = None
    _m.set_axon_ntff_profile_hook = lambda h: setattr(_m, "_hook", h)
    _m.get_axon_ntff_profile_hook = lambda: _m._hook
    _sys.modules["antenv.axon_hooks"] = _m
    _antenv.axon_hooks = _m

import concourse.bass as bass
from concourse import mybir
from concourse.bass_utils import run_bass_kernel_spmd
from concourse.tile import TileContext
from concourse.library_config import mlp as mlp_lib
from concourse.library_overlay import lower_extended_insts
# ---------------------------------------------------------------------------
# Workarounds for this walrus build (see notes below), self-contained.
# ---------------------------------------------------------------------------


def _split_multiwait(nc):
    """This walrus build rejects >1 sync-wait per instruction ("Too many sync
    wait commands").  Hoist extra SyncWaits onto NoOps inserted immediately
    before the instruction on the same engine (sequencer executes them in
    order, so cumulative wait semantics are unchanged)."""
    uid = 0
    for f in nc.m.functions:
        for b in f.blocks:
            il = b.instructions
            i = 0
            while i < len(il):
                inst = il[i]
                si = inst.sync_info
                if si is not None and si.on_wait and len(si.on_wait) > 1:
                    waits = list(si.on_wait)
                    si.on_wait = waits[-1:]
                    for w in waits[:-1]:
                        uid += 1
                        nop = mybir.InstNoOp(name=f"I-mwsplit-{uid}", ins=[], outs=[])
                        nop.engine = inst.engine
                        nop.sync_info = mybir.SyncInfo(on_wait=[w], on_update=[])
                        il.insert(i, nop)
                        i += 1
                i += 1


def _enable_dynamic_dma():
    """Without --dge-levels this walrus build logs "DynamicDMA is disabled"
    and silently compiles dynamic-AP DMAs as plain sequential copies."""
    from concourse import bass_utils as _bu

    if getattr(_bu.get_walrus_args, "_dyndma_patched", False):
        return
    _orig = _bu.get_walrus_args

    def _patched(arch, tmpdir, *, dve_root=None):
        return _orig(arch, tmpdir, dve_root=dve_root) + [
            "--dge-levels=vector_dynamic_offsets,scalar_dynamic_offset,dst_reduce"
        ]

    _patched._dyndma_patched = True
    _bu.get_walrus_args = _patched


_enable_dynamic_dma()


def _light_drain_and_barrier(self, tick_clock, wait_clock):
    """Tile teardown with sem-only engine barriers (saves ~2 us vs the
    full drain+barrier pair; waits split to 1/instruction for this walrus)."""
    from concourse.vector_clock import ScopedClock as _SC

    nc = self.nc
    probe = nc.sync.nop()
    wait_clock.add_sem_waits(probe.ins, _SC({None: tick_clock.global_clock}))
    si = probe.ins.sync_info
    waits = list(si.on_wait) if si is not None and si.on_wait else []
    if len(waits) > 1:
        si.on_wait = waits[:1]
        for w in waits[1:]:
            extra = nc.sync.nop()
            extra.ins.sync_info = mybir.SyncInfo(on_wait=[w], on_update=[])
    nc.sync.drain()
    nc.all_engine_barrier(sem_only=True)
    popped = nc._tile_sem_poison_stack.pop()
    assert popped is self._sem_poison
    nc.clear_and_free_semaphores(list(self.sems.allocated().values()))
    nc.all_engine_barrier(sem_only=True)


TileContext._drain_and_barrier = _light_drain_and_barrier

# ---------------------------------------------------------------------------
# Problem constants (hardcoded per the task spec).
# ---------------------------------------------------------------------------

B, C, K, DIM, VOCAB = 16384, 10, 8, 128, 100000
NCORES = 8
BL = B // NCORES  # 2048 examples per core
P = 128
T = BL // P  # 16 example slots per partition
CTX_ROWS = BL * C  # unique-ctx table rows (zero-padded), < 2^15
NEG_ROWS = BL * K  # unique-neg table rows (zero-padded)
NQ = 4  # SWDGE queues
F32 = mybir.dt.float32
I16 = mybir.dt.int16
EMB = mybir.dt.bfloat16
EMB8 = mybir.dt.float8e4
TAB_DT = ml_dtypes.bfloat16
TAB8_DT = mybir.dt.np(EMB8)

# Dense-stream staircases: m_j = number of examples whose j-th (sorted)
# row comes from the dense stream.  Multiples of 128.  Biased dense-heavy
# (beyond the single-occurrence means: ctx Bin(10,.815)->8.15, neg
# Bin(8,.849)->6.79): the extra dense slots are dup-filled (~2% more HBM
# bytes) but shrink the serial GPSIMD gather tail that gates the PSUM
# pooling and with it the whole DVE back half.
M_CTX = (2048,) * 6 + (1920, 1664, 1536, 1536)
M_NEG = (2048,) * 5 + (1792, 1536, 1536)
# Fully-dense levels (m == BL) stream in fp8-e4m3: the ctx tiles feed the
# PE pooling matmul directly (fp8 is a native matmul operand dtype); the
# neg tiles are upcast to bf16 on the idle Scalar engine before the DVE
# muls.  Mixed dense+gather levels stay bf16 (dma_gather rows must be a
# multiple of 256B, and one tile has one dtype).  Simulated end-to-end
# rel err of fp8 dense on both sides: ~8.6e-4 vs the 2e-2 gate.
CTX_N8 = sum(1 for m in M_CTX if m == BL)  # 6 fully-dense ctx levels
CTXA8_ROWS = sum(M_CTX)  # 18944: ALL ctx dense rows stream fp8
# Neg tiles stay bf16: a mixed bf16 x fp8 DVE mul measured at HALF the
# bf16 x bf16 rate, costing more than the fp8 DMA bytes saved.
NEGA_ROWS = sum(M_NEG)  # 15104 bf16 neg dense rows
CTX_G = [BL - m for m in M_CTX]  # per-position gather counts
NEG_G = [BL - m for m in M_NEG]
CTX_GCOLS = sum(g // 16 for g in CTX_G)  # 248 idx cols
NEG_GCOLS = sum(g // 16 for g in NEG_G)  # 160 idx cols
GCOLS = CTX_GCOLS + NEG_GCOLS
KH = K // 2
QW = 512  # PSUM bank width in f32: pooling/mul run in 4 column quarters

_cached_nc = None


def _build():
    global _cached_nc
    if _cached_nc is not None:
        return _cached_nc
    _orig_aeb = bass.Bass.all_engine_barrier

    def _semonly_aeb(self, *, sem_only=False):
        return _orig_aeb(self, sem_only=True)

    bass.Bass.all_engine_barrier = _semonly_aeb
    try:
        nc = bass.Bass(num_swdge_queues=NQ)
    finally:
        bass.Bass.all_engine_barrier = _orig_aeb

    ctx_tab = nc.declare_dram_parameter("ctx_tab", [CTX_ROWS, DIM], EMB, isOutput=False)
    neg_tab = nc.declare_dram_parameter("neg_tab", [NEG_ROWS, DIM], EMB, isOutput=False)
    ctxA8 = nc.declare_dram_parameter("ctxA8", [CTXA8_ROWS, DIM], EMB8, isOutput=False)
    negA = nc.declare_dram_parameter("negA", [NEGA_ROWS, DIM], EMB, isOutput=False)
    gidx = nc.declare_dram_parameter("gidx", [P, GCOLS], I16, isOutput=False)
    ident_d = nc.declare_dram_parameter("ident", [P, P], EMB, isOutput=False)
    ident8_d = nc.declare_dram_parameter("ident8", [P, P], EMB8, isOutput=False)
    # wm cols [0, K*T), labels cols [K*T, 2*K*T)
    wml = nc.declare_dram_parameter("wml", [P, 2 * K * T], F32, isOutput=False)
    out = nc.declare_dram_parameter("out", [P, 2 * T], F32, isOutput=True)

    # Issue the library reload in the main block, before the Tile preamble:
    # the Q7 ucode load (~11 us) then overlaps the EVSEM startup barriers.
    nc.gpsimd.load_library(mlp_lib)

    with TileContext(nc) as tc:
        with (
            tc.tile_pool(name="idxp", bufs=1) as idxp,
            tc.tile_pool(name="tiles", bufs=1) as tiles,
            tc.psum_pool(name="ps", bufs=1) as psp,
            tc.tile_pool(name="prod", bufs=3) as prodp,
            tc.tile_pool(name="fold", bufs=3) as foldp,
            tc.tile_pool(name="epi", bufs=1) as epip,
        ):
            # --- idx / wml loads (SP queue, small) -----------------------
            gidx_sb = idxp.tile([P, GCOLS], I16, tag="gidx", name="gidx")
            wml_sb = idxp.tile([P, 2 * K * T], F32)

            # --- position / k tiles --------------------------------------
            # ALL ctx dense data is fp8 (it only feeds the PE pooling
            # matmul).  For staircase levels (c>=CTX_N8) the dense prefix
            # and the gathered tail live in SEPARATE tiles: the gather tail
            # is bf16 (dma_gather rows must be 256B-aligned), and the split
            # means the PSUM chains for quarters 0-2 never depend on any
            # gather — only the q3 chain touches the bf16 gather tiles, via
            # column-subrange matmuls.
            ctx_t = [
                tiles.tile([P, BL], EMB8, tag=f"ct{c}", name=f"ct{c}")
                for c in range(CTX_N8)
            ]
            ctxd_t, ctxg_t = {}, {}
            for c in range(CTX_N8, C):
                mt = M_CTX[c] // P
                ctxd_t[c] = tiles.tile(
                    [P, mt * DIM], EMB8, tag=f"cd{c}", name=f"cd{c}"
                )
                ctxg_t[c] = tiles.tile(
                    [P, (T - mt) * DIM], EMB, tag=f"cg{c}", name=f"cg{c}"
                )
            # k0-3 are individual tiles; k4-7 PAIR into two double-width
            # tiles so their DVE mul/fold/reduce each merge two k's into
            # one instruction (saves ~0.4us of fixed cost per merged op).
            NPAIR0 = 4  # first paired k
            neg_t = [
                tiles.tile([P, BL], EMB, tag=f"nt{k}", name=f"nt{k}")
                for k in range(NPAIR0)
            ]
            np_t = [
                tiles.tile([P, 2 * BL], EMB, tag=f"npr{j}", name=f"npr{j}")
                for j in range((K - NPAIR0) // 2)
            ]

            def _negv(k):
                if k < NPAIR0:
                    return neg_t[k][:]
                j, s = (k - NPAIR0) // 2, (k - NPAIR0) % 2
                return np_t[j][:, s * BL : (s + 1) * BL]

            # identity weights for the PE pooling copies come from DRAM so
            # the GPSIMD stream stays free for the gathers
            ident = idxp.tile([P, P], EMB, tag="ident", name="ident")
            ident8 = idxp.tile([P, P], EMB8, tag="ident8", name="ident8")
            # Queue order matters: the ctx tiles gate the PE chains and with
            # them the whole DVE back half, so they go FIRST on both HWDGE
            # queues, after only the tiny weights/idx tiles each consumer
            # needs early (ident8 for the PE, gidx for the gathers).  wml is
            # only read by the epilogue and loads last.
            nc.sync.dma_start(out=ident8[:], in_=ident8_d[:])
            nc.scalar.dma_start(out=ident[:], in_=ident_d[:])

            # --- dense streams (HWDGE queues) ---------------------------
            # A is packed p-major host-side: row r of level j holds the row
            # for slot (p = r // mt, t = r % mt), so each descriptor moves a
            # contiguous mt-row span into one partition.  c0/c1 go ahead of
            # even gidx: the PE chains start on c0's arrival, while the
            # gathers (behind the ~15us GPSIMD library load) don't need
            # gidx until later.  wml is only read by the epilogue and loads
            # last.
            offs = [sum(M_CTX[:c]) for c in range(C)]
            for c in range(C):
                m = M_CTX[c]
                dst_tile = ctx_t[c] if c < CTX_N8 else ctxd_t[c]
                eng = nc.sync if c % 2 == 0 else nc.scalar
                eng.dma_start(
                    out=dst_tile[:].rearrange("p (t d) -> p t d", d=DIM),
                    in_=ctxA8[offs[c] : offs[c] + m, :].rearrange(
                        "(p t) d -> p t d", p=P
                    ),
                )
                if c == 1:
                    nc.sync.dma_start(out=gidx_sb[:], in_=gidx[:])
            noffs = [sum(M_NEG[:k]) for k in range(K)]
            for k in range(2):
                m = M_NEG[k]
                eng = nc.sync if k % 2 == 0 else nc.scalar
                eng.dma_start(
                    out=neg_t[k][:].rearrange("p (t d) -> p t d", d=DIM)[
                        :, : m // P, :
                    ],
                    in_=negA[noffs[k] : noffs[k] + m, :].rearrange(
                        "(p t) d -> p t d", p=P
                    ),
                )
            nc.scalar.dma_start(out=wml_sb[:], in_=wml[:])

            # --- gathers next: they serialize on GPSIMD.  Ctx first (they
            # gate the PSUM pooling and with it the whole DVE back half),
            # largest first within each side; queue round-robin.
            regs = {}
            for n in sorted({*CTX_G, *NEG_G} - {0}):
                regs[n] = nc.gpsimd.to_reg(n)
            idx_off = {}
            off = 0
            for c in range(C):
                if CTX_G[c]:
                    idx_off[("c", c)] = off
                    off += CTX_G[c] // 16
            for k in range(K):
                if NEG_G[k]:
                    idx_off[("n", k)] = off
                    off += NEG_G[k] // 16
            order = sorted(
                idx_off,
                key=lambda s: (
                    s[0] != "c",
                    -(CTX_G[s[1]] if s[0] == "c" else NEG_G[s[1]]),
                ),
            )
            # queues 1-3 only: queue 0 carries the Pool-static neg dense
            # streams, whose packets would contend with the gather drain
            for qn, key in enumerate(order):
                side, j = key
                g = CTX_G[j] if side == "c" else NEG_G[j]
                if side == "c":
                    out_ap = ctxg_t[j][:].rearrange("p (t d) -> p t d", d=DIM)
                else:
                    m = M_NEG[j]
                    out_ap = _negv(j).rearrange("p (t d) -> p t d", d=DIM)[
                        :, m // P :, :
                    ]
                tab = ctx_tab if side == "c" else neg_tab
                o = idx_off[key]
                nc.gpsimd.dma_gather(
                    out_ap,
                    tab[:],
                    gidx_sb[:, o : o + g // 16],
                    g, regs[g], DIM,
                    single_packet=False,
                    queue_num=1 + qn % (NQ - 1),
                )

            # --- neg k2-7 as SWDGE static DMAs behind the gathers --------
            # (their ~2.9MB stays off the HWDGE queues that must deliver
            # the ctx tiles early; the DVE consumes them late)
            for k in range(2, K):
                m = M_NEG[k]
                nc.gpsimd.dma_start(
                    out=_negv(k).rearrange("p (t d) -> p t d", d=DIM)[
                        :, : m // P, :
                    ],
                    in_=negA[noffs[k] : noffs[k] + m, :].rearrange(
                        "(p t) d -> p t d", p=P
                    ),
                )

            # --- ctx pooling on the PE: identity matmuls accumulate the 10
            # position tiles into PSUM f32, one 512-col quarter per bank.
            acc_ps = [
                psp.tile([P, QW], F32, tag=f"acc{q}", name=f"acc{q}")
                for q in range(BL // QW)
            ]
            # quarters 0-2 read only fp8 dense tiles (no gather deps); the
            # q3 chain (cols [1536, 2048)) mixes per-c dense/gather column
            # subranges — every column is first written by c0's start=True
            # full-width matmul, so later subrange matmuls all accumulate.
            NQ3 = (BL // QW) - 1
            for q in range(NQ3):
                for c in range(C):
                    rhs_tile = ctx_t[c] if c < CTX_N8 else ctxd_t[c]
                    nc.tensor.matmul(
                        out=acc_ps[q][:],
                        lhsT=ident8[:],
                        rhs=rhs_tile[:, q * QW : (q + 1) * QW],
                        start=(c == 0),
                        stop=(c == C - 1),
                    )
            q3_parts = []  # (rhs_ap, out_lo, out_hi, fp8)
            for c in range(C):
                if c < CTX_N8:
                    q3_parts.append((ctx_t[c][:, NQ3 * QW :], 0, QW, True))
                else:
                    mt = M_CTX[c] // P
                    dcols = (mt - 12) * DIM  # dense cols within q3
                    if dcols > 0:
                        q3_parts.append(
                            (ctxd_t[c][:, 12 * DIM :], 0, dcols, True)
                        )
                    q3_parts.append((ctxg_t[c][:], dcols, QW, False))
            for i, (rhs, lo, hi, fp8) in enumerate(q3_parts):
                nc.tensor.matmul(
                    out=acc_ps[NQ3][:, lo:hi],
                    lhsT=(ident8 if fp8 else ident)[:],
                    rhs=rhs,
                    start=(i == 0),
                    stop=(i == len(q3_parts) - 1),
                )

            # --- PSUM -> SBUF src_acc on the (idle) Scalar engine: DVE reads
            # PSUM at half rate, so the muls read an SBUF copy instead.
            src_acc = idxp.tile([P, BL], EMB, tag="srcacc", name="srcacc")
            for q in range(BL // QW):
                nc.scalar.activation(
                    out=src_acc[:, q * QW : (q + 1) * QW],
                    in_=acc_ps[q][:],
                    func=mybir.ActivationFunctionType.Copy,
                )

            # --- negatives: mul + fold + reduce on the DVE ----------------
            pred_halves = [
                epip.tile([P, KH * T], F32, tag="predlo", name="predlo"),
                epip.tile([P, KH * T], F32, tag="predhi", name="predhi"),
            ]
            # k0/k1 run in a 3/4 + 1/4 split: slots t<12 touch no gathered
            # data, so the DVE starts on them while the q3 (t>=12) gathers
            # and their PSUM chain are still in flight.  Remaining k's run
            # full-width (the per-op ~385ns DVE fixed cost makes finer
            # splits a net loss).
            TSPL = 12
            NSPLIT = 3  # how many leading k's run split
            prods, folds = {}, {}
            for k in range(NPAIR0):
                prods[k] = prodp.tile([P, BL], EMB, tag="prodch", name=f"prod{k}")
                folds[k] = foldp.tile([P, BL // 2], EMB, tag="foldch", name=f"fold{k}")

            def _piece(k, lo, hi):
                kk = k % KH
                pv = prods[k][:].rearrange("p (t d) -> p t d", d=DIM)
                fv = folds[k][:].rearrange("p (t d) -> p t d", d=DIM // 2)
                nc.vector.tensor_mul(
                    out=prods[k][:, lo * DIM : hi * DIM],
                    in0=src_acc[:, lo * DIM : hi * DIM],
                    in1=_negv(k)[:, lo * DIM : hi * DIM],
                )
                nc.vector.tensor_add(
                    out=fv[:, lo:hi, :],
                    in0=pv[:, lo:hi, : DIM // 2],
                    in1=pv[:, lo:hi, DIM // 2 :],
                )
                nc.vector.tensor_reduce(
                    out=pred_halves[k // KH][:, kk * T + lo : kk * T + hi],
                    in_=fv[:, lo:hi, :],
                    axis=mybir.AxisListType.X,
                    op=mybir.AluOpType.add,
                )

            for k in range(NSPLIT):
                _piece(k, 0, TSPL)
            for k in range(NSPLIT):
                _piece(k, TSPL, T)
            for k in range(NSPLIT, NPAIR0):
                _piece(k, 0, T)
            # paired k's: one double-width mul (src_acc broadcast over the
            # pair), one fold, one reduce — two k's per instruction
            for j in range((K - NPAIR0) // 2):
                k0p = NPAIR0 + 2 * j
                kk = k0p % KH
                prodp2 = prodp.tile(
                    [P, 2 * BL], EMB, tag="prodpair", name=f"prodp{j}"
                )
                foldp2 = foldp.tile(
                    [P, BL], EMB, tag="foldpair", name=f"foldp{j}"
                )
                nc.vector.tensor_mul(
                    out=prodp2[:].rearrange("p (o d) -> p o d", d=BL),
                    in0=src_acc[:]
                    .rearrange("p (o d) -> p o d", o=1)
                    .to_broadcast([P, 2, BL]),
                    in1=np_t[j][:].rearrange("p (o d) -> p o d", d=BL),
                )
                pv2 = prodp2[:].rearrange("p (t d) -> p t d", d=DIM)
                fv2 = foldp2[:].rearrange("p (t d) -> p t d", d=DIM // 2)
                nc.vector.tensor_add(
                    out=fv2, in0=pv2[:, :, : DIM // 2], in1=pv2[:, :, DIM // 2 :]
                )
                nc.vector.tensor_reduce(
                    out=pred_halves[k0p // KH][:, kk * T : (kk + 2) * T],
                    in_=fv2,
                    axis=mybir.AxisListType.X,
                    op=mybir.AluOpType.add,
                )

            # --- epilogue: wm * (softplus(pred) - pred*label), sum over K -
            # softplus composed as relu(x) + ln(1 + exp(-|x|)) (no softplus
            # ACT table in this build).  Done per k-half so the first half
            # overlaps the second half's reduces.
            for hh in range(2):
                pred = pred_halves[hh]
                wm = wml_sb[:, hh * KH * T : (hh + 1) * KH * T]
                lab = wml_sb[:, (K + hh * KH) * T : (K + (hh + 1) * KH) * T]
                sp_a = epip.tile([P, KH * T], F32, tag=f"spa{hh}", name=f"spa{hh}")
                nc.scalar.activation(
                    out=sp_a[:], in_=pred[:], func=mybir.ActivationFunctionType.Abs
                )
                nc.scalar.activation(
                    out=sp_a[:], in_=sp_a[:],
                    func=mybir.ActivationFunctionType.Exp, scale=-1.0,
                )
                nc.scalar.activation(
                    out=sp_a[:], in_=sp_a[:],
                    func=mybir.ActivationFunctionType.Ln, bias=1.0,
                )
                sp_r = epip.tile([P, KH * T], F32, tag=f"spr{hh}", name=f"spr{hh}")
                nc.scalar.activation(
                    out=sp_r[:], in_=pred[:], func=mybir.ActivationFunctionType.Relu
                )
                t1 = epip.tile([P, KH * T], F32, tag=f"t1{hh}", name=f"t1{hh}")
                nc.vector.tensor_mul(out=t1[:], in0=pred[:], in1=lab)
                nc.vector.tensor_sub(out=sp_r[:], in0=sp_r[:], in1=t1[:])
                nc.vector.tensor_add(out=sp_r[:], in0=sp_r[:], in1=sp_a[:])
                nc.vector.tensor_mul(out=sp_r[:], in0=sp_r[:], in1=wm)
                nh = epip.tile([P, T], F32, tag=f"nh{hh}", name=f"nh{hh}")
                nc.vector.tensor_reduce(
                    out=nh[:],
                    in_=sp_r[:].rearrange("p (k t) -> p t k", k=KH),
                    axis=mybir.AxisListType.X,
                    op=mybir.AluOpType.add,
                )
                # each half's output DMA overlaps the other half's epilogue
                nc.sync.dma_start(out=out[:, hh * T : (hh + 1) * T], in_=nh[:])

    _split_multiwait(nc)
    lower_extended_insts(nc)

    # Hoist the library reload to the very front of the main block: the
    # ~10 us Q7 ucode load then overlaps the Bass preamble (sem init +
    # all-core start barrier) instead of serializing after it.  The reload
    # has no register or semaphore operands, and the const MEMSETs are
    # native Pool ops (not library ucode), so reordering is safe.
    mainb = nc.m.functions[0].blocks[0]
    il = mainb.instructions
    reloads = [i for i in il if "Reload" in type(i).__name__
               or getattr(i, "op_name", "") == "PseudoReloadLibraryIndex"]
    for r in reloads:
        il.remove(r)
    for pos, r in enumerate(reloads):
        il.insert(pos, r)
    _cached_nc = nc
    return nc


def _wrap(flat):
    """[n] int16 (flat[i] gathers to out slot (i%128, i//128)) -> the
    dma_gather idx tile layout: [16, n//16] with (p, s) = flat[s*16+p],
    replicated to 128 partitions."""
    n = flat.shape[0]
    return np.tile(flat.reshape(n // 16, 16).T, (8, 1))


def _prep_side(vals, emb, m_levels, tab_rows, dense_dt=TAB_DT):
    """Shared host prep for one side (ctx or neg) of one core.

    vals: [BL, J] int64 vocab ids.  Returns (table, denseA, idx_np, order)
    where order is the per-example column permutation applied (singles
    first), so callers can permute slot-aligned payloads identically.
    """
    J = vals.shape[1]
    u, inv = np.unique(vals.ravel(), return_inverse=True)
    ids = inv.reshape(BL, J)
    cnt = np.bincount(inv)
    singles = cnt[ids] == 1
    order = np.argsort(~singles, axis=1, kind="stable")
    sids = np.take_along_axis(ids, order, axis=1)
    tab = np.zeros((tab_rows, DIM), dtype=TAB_DT)
    tab[: len(u)] = emb[u].astype(TAB_DT)
    # dense streams packed p-major per level: row r <-> slot (p=r//mt, t=r%mt)
    # fully-dense levels go to the fp8 stream, mixed levels to the bf16 one.
    segs = []
    for j in range(J):
        m = m_levels[j]
        segs.append(
            tab[sids[:m, j].reshape(m // P, P).T.ravel()].astype(dense_dt)
        )
    denseA = np.concatenate(segs, axis=0)
    gsegs = [
        _wrap(sids[m_levels[j] :, j].astype(np.int16))
        for j in range(J)
        if m_levels[j] < BL
    ]
    idx_np = np.concatenate(gsegs, axis=1)
    return tab, denseA, idx_np, order


def kernel(contexts, focus_word, weight_mask, labels, ctx_emb, neg_emb):
    contexts = np.asarray(contexts)
    focus_word = np.asarray(focus_word)
    weight_mask = np.asarray(weight_mask, dtype=np.float32)
    labels = np.asarray(labels, dtype=np.float32)
    ctx_emb = np.asarray(ctx_emb, dtype=np.float32)
    neg_emb = np.asarray(neg_emb, dtype=np.float32)

    nc = _build()

    in_maps = []
    dens = []
    for i in range(NCORES):
        sl = slice(i * BL, (i + 1) * BL)
        ctx_i = contexts[sl].astype(np.int64)  # [BL, C]
        foc_i = focus_word[sl].astype(np.int64)  # [BL, K]
        wm_i = weight_mask[sl]  # [BL, K]
        lab_i = labels[sl]

        ctab, ctxA8_np, ctx_idx_np, _ = _prep_side(
            ctx_i, ctx_emb, M_CTX, CTX_ROWS, dense_dt=TAB8_DT
        )
        ntab, negA_np, neg_idx_np, n_order = _prep_side(
            foc_i, neg_emb, M_NEG, NEG_ROWS
        )
        # negative slots were permuted per-example: permute wm/labels too
        wm_s = np.take_along_axis(wm_i, n_order, axis=1)
        lab_s = np.take_along_axis(lab_i, n_order, axis=1)

        # wm/lab to [P, K*T]: (p, k*T+t) = value[e = t*128+p, k]
        wm_r = wm_s.reshape(T, P, K).transpose(1, 2, 0).reshape(P, K * T)
        lab_r = lab_s.reshape(T, P, K).transpose(1, 2, 0).reshape(P, K * T)
        wml_np = np.concatenate([wm_r, lab_r], axis=1)

        in_maps.append(
            {
                "ctx_tab": ctab,
                "neg_tab": ntab,
                "ctxA8": np.ascontiguousarray(ctxA8_np),
                "negA": np.ascontiguousarray(negA_np),
                "gidx": np.ascontiguousarray(
                    np.concatenate([ctx_idx_np, neg_idx_np], axis=1)
                ),
                "ident": np.eye(P, dtype=TAB_DT),
                "ident8": np.eye(P, dtype=TAB8_DT),
                "wml": np.ascontiguousarray(wml_np),
            }
        )
        dens.append(wm_i.sum(axis=1))  # [BL] row denominators

    res = run_bass_kernel_spmd(nc, in_maps, core_ids=list(range(NCORES)))

    total = 0.0
    for i in range(NCORES):
        o = res.results[i]["out"]  # [P, 2T]: two K-half numerators
        num = o[:, :T] + o[:, T:]
        num_e = num.T.reshape(BL)  # [BL] in example order
        total += float((num_e.astype(np.float64) / dens[i].astype(np.float64)).sum())
    return np.float32(total / B)


# revision 62
# speedup vs baseline: 1.0587x; 1.0587x over previous
"""CBOW forward (embedding lookup + pooled dot + weighted BCE) on 8 TRN2 cores.

Strategy: data-parallel over the batch; each core owns BL = 2048 examples.

The original kernel was bound by SWDGE descriptor generation: every gathered
row costs ~2.4 ns of serial GPSIMD (Pool engine) ucode time, and 36864
rows/core -> ~89 us.  This version removes ~80% of that by exploiting a
layout freedom: within an example, context positions (and negative slots,
together with their weight/label) are interchangeable.  Host-side we order
each example's rows so that rows which can be served by a *statically
shaped* dense stream come first:

  - For position/slot index j, a static "staircase" m_j says examples
    0..m_j-1 get row j from a dense stream tensor (packed host-side in
    exactly slot order), and examples >= m_j get it via dma_gather.
  - Each example's single-occurrence rows are placed in the dense slots
    first, so the dense stream is (nearly) duplication-free: it is a
    permutation of unique table rows, plus a few % duplicates where an
    example has fewer single-occurrence rows than dense slots.

Dense streams ride plain static DMAs on the two HWDGE queues (SP/ACT
engines), packed p-major host-side so each descriptor covers a 16-row
4 KB span (HWDGE charges ~1 ns/descriptor to the issuing engine).  Only
~6.5k rows/core (the staircase tails) still use dma_gather.

Context pooling runs on the otherwise-idle TensorEngine: identity-weight
matmuls accumulate the 10 position tiles into PSUM (f32), freeing the DVE
for the negative phase (mul + fold + reduce) and epilogue.

Tables are bf16; reduction and epilogue stay f32.
Host: per_row = num / sum_k(weight_mask); answer = mean over all rows.
"""

import numpy as np
import ml_dtypes

# run_bass_kernel_spmd under axon imports antenv.axon_hooks unconditionally;
# provide an in-process stub if the container image lacks that module.
import sys as _sys
import types as _types

try:
    import antenv.axon_hooks  # noqa: F401
except Exception:
    import antenv as _antenv

    _m = _types.ModuleType("antenv.axon_hooks")
    _m._hook = None
    _m.set_axon_ntff_profile_hook = lambda h: setattr(_m, "_hook", h)
    _m.get_axon_ntff_profile_hook = lambda: _m._hook
    _sys.modules["antenv.axon_hooks"] = _m
    _antenv.axon_hooks = _m

import concourse.bass as bass
from concourse import mybir
from concourse.bass_utils import run_bass_kernel_spmd
from concourse.tile import TileContext
from concourse.library_config import mlp as mlp_lib
from concourse.library_overlay import lower_extended_insts
# ---------------------------------------------------------------------------
# Workarounds for this walrus build (see notes below), self-contained.
# ---------------------------------------------------------------------------


def _split_multiwait(nc):
    """This walrus build rejects >1 sync-wait per instruction ("Too many sync
    wait commands").  Hoist extra SyncWaits onto NoOps inserted immediately
    before the instruction on the same engine (sequencer executes them in
    order, so cumulative wait semantics are unchanged)."""
    uid = 0
    for f in nc.m.functions:
        for b in f.blocks:
            il = b.instructions
            i = 0
            while i < len(il):
                inst = il[i]
                si = inst.sync_info
                if si is not None and si.on_wait and len(si.on_wait) > 1:
                    waits = list(si.on_wait)
                    si.on_wait = waits[-1:]
                    for w in waits[:-1]:
                        uid += 1
                        nop = mybir.InstNoOp(name=f"I-mwsplit-{uid}", ins=[], outs=[])
                        nop.engine = inst.engine
                        nop.sync_info = mybir.SyncInfo(on_wait=[w], on_update=[])
                        il.insert(i, nop)
                        i += 1
                i += 1


def _enable_dynamic_dma():
    """Without --dge-levels this walrus build logs "DynamicDMA is disabled"
    and silently compiles dynamic-AP DMAs as plain sequential copies."""
    from concourse import bass_utils as _bu

    if getattr(_bu.get_walrus_args, "_dyndma_patched", False):
        return
    _orig = _bu.get_walrus_args

    def _patched(arch, tmpdir, *, dve_root=None):
        return _orig(arch, tmpdir, dve_root=dve_root) + [
            "--dge-levels=vector_dynamic_offsets,scalar_dynamic_offset,dst_reduce"
        ]

    _patched._dyndma_patched = True
    _bu.get_walrus_args = _patched


_enable_dynamic_dma()


def _light_drain_and_barrier(self, tick_clock, wait_clock):
    """Tile teardown with sem-only engine barriers (saves ~2 us vs the
    full drain+barrier pair; waits split to 1/instruction for this walrus)."""
    from concourse.vector_clock import ScopedClock as _SC

    nc = self.nc
    probe = nc.sync.nop()
    wait_clock.add_sem_waits(probe.ins, _SC({None: tick_clock.global_clock}))
    si = probe.ins.sync_info
    waits = list(si.on_wait) if si is not None and si.on_wait else []
    if len(waits) > 1:
        si.on_wait = waits[:1]
        for w in waits[1:]:
            extra = nc.sync.nop()
            extra.ins.sync_info = mybir.SyncInfo(on_wait=[w], on_update=[])
    nc.sync.drain()
    nc.all_engine_barrier(sem_only=True)
    popped = nc._tile_sem_poison_stack.pop()
    assert popped is self._sem_poison
    nc.clear_and_free_semaphores(list(self.sems.allocated().values()))
    nc.all_engine_barrier(sem_only=True)


TileContext._drain_and_barrier = _light_drain_and_barrier

# ---------------------------------------------------------------------------
# Problem constants (hardcoded per the task spec).
# ---------------------------------------------------------------------------

B, C, K, DIM, VOCAB = 16384, 10, 8, 128, 100000
NCORES = 8
BL = B // NCORES  # 2048 examples per core
P = 128
T = BL // P  # 16 example slots per partition
CTX_ROWS = BL * C  # unique-ctx table rows (zero-padded), < 2^15
NEG_ROWS = BL * K  # unique-neg table rows (zero-padded)
NQ = 4  # SWDGE queues
F32 = mybir.dt.float32
I16 = mybir.dt.int16
EMB = mybir.dt.bfloat16
EMB8 = mybir.dt.float8e4
TAB_DT = ml_dtypes.bfloat16
TAB8_DT = mybir.dt.np(EMB8)

# Dense-stream staircases: m_j = number of examples whose j-th (sorted)
# row comes from the dense stream.  Multiples of 128.  Biased dense-heavy
# (beyond the single-occurrence means: ctx Bin(10,.815)->8.15, neg
# Bin(8,.849)->6.79): the extra dense slots are dup-filled (~2% more HBM
# bytes) but shrink the serial GPSIMD gather tail that gates the PSUM
# pooling and with it the whole DVE back half.
M_CTX = (2048,) * 6 + (1920, 1664, 1536, 1536)
M_NEG = (2048,) * 5 + (1792, 1536, 1536)
# Fully-dense levels (m == BL) stream in fp8-e4m3: the ctx tiles feed the
# PE pooling matmul directly (fp8 is a native matmul operand dtype); the
# neg tiles are upcast to bf16 on the idle Scalar engine before the DVE
# muls.  Mixed dense+gather levels stay bf16 (dma_gather rows must be a
# multiple of 256B, and one tile has one dtype).  Simulated end-to-end
# rel err of fp8 dense on both sides: ~8.6e-4 vs the 2e-2 gate.
CTX_N8 = sum(1 for m in M_CTX if m == BL)  # 6 fully-dense ctx levels
CTXA8_ROWS = sum(M_CTX)  # 18944: ALL ctx dense rows stream fp8
# Neg tiles stay bf16: a mixed bf16 x fp8 DVE mul measured at HALF the
# bf16 x bf16 rate, costing more than the fp8 DMA bytes saved.
NEGA_ROWS = sum(M_NEG)  # 15104 bf16 neg dense rows
CTX_G = [BL - m for m in M_CTX]  # per-position gather counts
NEG_G = [BL - m for m in M_NEG]
CTX_GCOLS = sum(g // 16 for g in CTX_G)  # 248 idx cols
NEG_GCOLS = sum(g // 16 for g in NEG_G)  # 160 idx cols
GCOLS = CTX_GCOLS + NEG_GCOLS
KH = K // 2
QW = 512  # PSUM bank width in f32: pooling/mul run in 4 column quarters

_cached_nc = None


def _build():
    global _cached_nc
    if _cached_nc is not None:
        return _cached_nc
    _orig_aeb = bass.Bass.all_engine_barrier

    def _semonly_aeb(self, *, sem_only=False):
        return _orig_aeb(self, sem_only=True)

    bass.Bass.all_engine_barrier = _semonly_aeb
    try:
        nc = bass.Bass(num_swdge_queues=NQ)
    finally:
        bass.Bass.all_engine_barrier = _orig_aeb

    ctx_tab = nc.declare_dram_parameter("ctx_tab", [CTX_ROWS, DIM], EMB, isOutput=False)
    neg_tab = nc.declare_dram_parameter("neg_tab", [NEG_ROWS, DIM], EMB, isOutput=False)
    ctxA8 = nc.declare_dram_parameter("ctxA8", [CTXA8_ROWS, DIM], EMB8, isOutput=False)
    negA = nc.declare_dram_parameter("negA", [NEGA_ROWS, DIM], EMB, isOutput=False)
    gidx = nc.declare_dram_parameter("gidx", [P, GCOLS], I16, isOutput=False)
    ident_d = nc.declare_dram_parameter("ident", [P, P], EMB, isOutput=False)
    ident8_d = nc.declare_dram_parameter("ident8", [P, P], EMB8, isOutput=False)
    # wm cols [0, K*T), labels cols [K*T, 2*K*T)
    wml = nc.declare_dram_parameter("wml", [P, 2 * K * T], F32, isOutput=False)
    out = nc.declare_dram_parameter("out", [P, 2 * T], F32, isOutput=True)

    # Issue the library reload in the main block, before the Tile preamble:
    # the Q7 ucode load (~11 us) then overlaps the EVSEM startup barriers.
    nc.gpsimd.load_library(mlp_lib)

    with TileContext(nc) as tc:
        with (
            tc.tile_pool(name="idxp", bufs=1) as idxp,
            tc.tile_pool(name="tiles", bufs=1) as tiles,
            tc.psum_pool(name="ps", bufs=1) as psp,
            tc.tile_pool(name="prod", bufs=3) as prodp,
            tc.tile_pool(name="fold", bufs=3) as foldp,
            tc.tile_pool(name="epi", bufs=1) as epip,
        ):
            # --- idx / wml loads (SP queue, small) -----------------------
            gidx_sb = idxp.tile([P, GCOLS], I16, tag="gidx", name="gidx")
            wml_sb = idxp.tile([P, 2 * K * T], F32)

            # --- position / k tiles --------------------------------------
            # ALL ctx dense data is fp8 (it only feeds the PE pooling
            # matmul).  For staircase levels (c>=CTX_N8) the dense prefix
            # and the gathered tail live in SEPARATE tiles: the gather tail
            # is bf16 (dma_gather rows must be 256B-aligned), and the split
            # means the PSUM chains for quarters 0-2 never depend on any
            # gather — only the q3 chain touches the bf16 gather tiles, via
            # column-subrange matmuls.
            ctx_t = [
                tiles.tile([P, BL], EMB8, tag=f"ct{c}", name=f"ct{c}")
                for c in range(CTX_N8)
            ]
            ctxd_t, ctxg_t = {}, {}
            for c in range(CTX_N8, C):
                mt = M_CTX[c] // P
                ctxd_t[c] = tiles.tile(
                    [P, mt * DIM], EMB8, tag=f"cd{c}", name=f"cd{c}"
                )
                ctxg_t[c] = tiles.tile(
                    [P, (T - mt) * DIM], EMB, tag=f"cg{c}", name=f"cg{c}"
                )
            neg_t = [
                tiles.tile([P, BL], EMB, tag=f"nt{k}", name=f"nt{k}") for k in range(K)
            ]

            # identity weights for the PE pooling copies come from DRAM so
            # the GPSIMD stream stays free for the gathers
            ident = idxp.tile([P, P], EMB, tag="ident", name="ident")
            ident8 = idxp.tile([P, P], EMB8, tag="ident8", name="ident8")
            # Queue order matters: the ctx tiles gate the PE chains and with
            # them the whole DVE back half, so they go FIRST on both HWDGE
            # queues, after only the tiny weights/idx tiles each consumer
            # needs early (ident8 for the PE, gidx for the gathers).  wml is
            # only read by the epilogue and loads last.
            nc.sync.dma_start(out=ident8[:], in_=ident8_d[:])
            nc.scalar.dma_start(out=ident[:], in_=ident_d[:])

            # --- dense streams (HWDGE queues) ---------------------------
            # A is packed p-major host-side: row r of level j holds the row
            # for slot (p = r // mt, t = r % mt), so each descriptor moves a
            # contiguous mt-row span into one partition.  c0/c1 go ahead of
            # even gidx: the PE chains start on c0's arrival, while the
            # gathers (behind the ~15us GPSIMD library load) don't need
            # gidx until later.  wml is only read by the epilogue and loads
            # last.
            offs = [sum(M_CTX[:c]) for c in range(C)]
            for c in range(C):
                m = M_CTX[c]
                dst_tile = ctx_t[c] if c < CTX_N8 else ctxd_t[c]
                eng = nc.sync if c % 2 == 0 else nc.scalar
                eng.dma_start(
                    out=dst_tile[:].rearrange("p (t d) -> p t d", d=DIM),
                    in_=ctxA8[offs[c] : offs[c] + m, :].rearrange(
                        "(p t) d -> p t d", p=P
                    ),
                )
                if c == 1:
                    nc.sync.dma_start(out=gidx_sb[:], in_=gidx[:])
            noffs = [sum(M_NEG[:k]) for k in range(K)]
            for k in range(2):
                m = M_NEG[k]
                eng = nc.sync if k % 2 == 0 else nc.scalar
                eng.dma_start(
                    out=neg_t[k][:].rearrange("p (t d) -> p t d", d=DIM)[
                        :, : m // P, :
                    ],
                    in_=negA[noffs[k] : noffs[k] + m, :].rearrange(
                        "(p t) d -> p t d", p=P
                    ),
                )
            nc.scalar.dma_start(out=wml_sb[:], in_=wml[:])

            # --- gathers next: they serialize on GPSIMD.  Ctx first (they
            # gate the PSUM pooling and with it the whole DVE back half),
            # largest first within each side; queue round-robin.
            regs = {}
            for n in sorted({*CTX_G, *NEG_G} - {0}):
                regs[n] = nc.gpsimd.to_reg(n)
            idx_off = {}
            off = 0
            for c in range(C):
                if CTX_G[c]:
                    idx_off[("c", c)] = off
                    off += CTX_G[c] // 16
            for k in range(K):
                if NEG_G[k]:
                    idx_off[("n", k)] = off
                    off += NEG_G[k] // 16
            order = sorted(
                idx_off,
                key=lambda s: (
                    s[0] != "c",
                    -(CTX_G[s[1]] if s[0] == "c" else NEG_G[s[1]]),
                ),
            )
            # queues 1-3 only: queue 0 carries the Pool-static neg dense
            # streams, whose packets would contend with the gather drain
            for qn, key in enumerate(order):
                side, j = key
                g = CTX_G[j] if side == "c" else NEG_G[j]
                if side == "c":
                    out_ap = ctxg_t[j][:].rearrange("p (t d) -> p t d", d=DIM)
                else:
                    m = M_NEG[j]
                    out_ap = neg_t[j][:].rearrange("p (t d) -> p t d", d=DIM)[
                        :, m // P :, :
                    ]
                tab = ctx_tab if side == "c" else neg_tab
                o = idx_off[key]
                nc.gpsimd.dma_gather(
                    out_ap,
                    tab[:],
                    gidx_sb[:, o : o + g // 16],
                    g, regs[g], DIM,
                    single_packet=False,
                    queue_num=1 + qn % (NQ - 1),
                )

            # --- neg k2-7 as SWDGE static DMAs behind the gathers --------
            # (their ~2.9MB stays off the HWDGE queues that must deliver
            # the ctx tiles early; the DVE consumes them late)
            for k in range(2, K):
                m = M_NEG[k]
                nc.gpsimd.dma_start(
                    out=neg_t[k][:].rearrange("p (t d) -> p t d", d=DIM)[
                        :, : m // P, :
                    ],
                    in_=negA[noffs[k] : noffs[k] + m, :].rearrange(
                        "(p t) d -> p t d", p=P
                    ),
                )

            # --- ctx pooling on the PE: identity matmuls accumulate the 10
            # position tiles into PSUM f32, one 512-col quarter per bank.
            acc_ps = [
                psp.tile([P, QW], F32, tag=f"acc{q}", name=f"acc{q}")
                for q in range(BL // QW)
            ]
            # The DVE is idle until the first src_acc copies land, so it
            # pre-sums c0+c1 (fp8+fp8 -> bf16) into S, dropping every PE
            # chain from 10 to 9 matmuls.  S goes LAST in each chain so the
            # early matmuls aren't gated on the DVE add.
            presum = idxp.tile([P, BL], EMB, tag="presum", name="presum")
            nc.vector.tensor_add(out=presum[:], in0=ctx_t[0][:], in1=ctx_t[1][:])

            # quarters 0-2 read only fp8 dense tiles (no gather deps); the
            # q3 chain (cols [1536, 2048)) mixes per-c dense/gather column
            # subranges — every column is first written by the start=True
            # full-width matmul, so later subrange matmuls all accumulate.
            NQ3 = (BL // QW) - 1
            for q in range(NQ3):
                chain = [
                    (ctx_t[c] if c < CTX_N8 else ctxd_t[c], True)
                    for c in range(2, C)
                ] + [(presum, False)]
                for i, (rhs_tile, fp8) in enumerate(chain):
                    nc.tensor.matmul(
                        out=acc_ps[q][:],
                        lhsT=(ident8 if fp8 else ident)[:],
                        rhs=rhs_tile[:, q * QW : (q + 1) * QW],
                        start=(i == 0),
                        stop=(i == len(chain) - 1),
                    )
            q3_parts = []  # (rhs_ap, out_lo, out_hi, fp8)
            for c in range(2, C):
                if c < CTX_N8:
                    q3_parts.append((ctx_t[c][:, NQ3 * QW :], 0, QW, True))
                else:
                    mt = M_CTX[c] // P
                    dcols = (mt - 12) * DIM  # dense cols within q3
                    if dcols > 0:
                        q3_parts.append(
                            (ctxd_t[c][:, 12 * DIM :], 0, dcols, True)
                        )
                    q3_parts.append((ctxg_t[c][:], dcols, QW, False))
            q3_parts.append((presum[:, NQ3 * QW :], 0, QW, False))
            for i, (rhs, lo, hi, fp8) in enumerate(q3_parts):
                nc.tensor.matmul(
                    out=acc_ps[NQ3][:, lo:hi],
                    lhsT=(ident8 if fp8 else ident)[:],
                    rhs=rhs,
                    start=(i == 0),
                    stop=(i == len(q3_parts) - 1),
                )

            # --- PSUM -> SBUF src_acc on the (idle) Scalar engine: DVE reads
            # PSUM at half rate, so the muls read an SBUF copy instead.
            src_acc = idxp.tile([P, BL], EMB, tag="srcacc", name="srcacc")
            for q in range(BL // QW):
                nc.scalar.activation(
                    out=src_acc[:, q * QW : (q + 1) * QW],
                    in_=acc_ps[q][:],
                    func=mybir.ActivationFunctionType.Copy,
                )

            # --- negatives: mul + fold + reduce on the DVE ----------------
            pred_halves = [
                epip.tile([P, KH * T], F32, tag="predlo", name="predlo"),
                epip.tile([P, KH * T], F32, tag="predhi", name="predhi"),
            ]
            # k0/k1 run in a 3/4 + 1/4 split: slots t<12 touch no gathered
            # data, so the DVE starts on them while the q3 (t>=12) gathers
            # and their PSUM chain are still in flight.  Remaining k's run
            # full-width (the per-op ~385ns DVE fixed cost makes finer
            # splits a net loss).
            TSPL = 12
            NSPLIT = 3  # how many leading k's run split
            prods, folds = {}, {}
            for k in range(K):
                prods[k] = prodp.tile([P, BL], EMB, tag="prodch", name=f"prod{k}")
                folds[k] = foldp.tile([P, BL // 2], EMB, tag="foldch", name=f"fold{k}")

            def _piece(k, lo, hi):
                kk = k % KH
                pv = prods[k][:].rearrange("p (t d) -> p t d", d=DIM)
                fv = folds[k][:].rearrange("p (t d) -> p t d", d=DIM // 2)
                nc.vector.tensor_mul(
                    out=prods[k][:, lo * DIM : hi * DIM],
                    in0=src_acc[:, lo * DIM : hi * DIM],
                    in1=neg_t[k][:, lo * DIM : hi * DIM],
                )
                nc.vector.tensor_add(
                    out=fv[:, lo:hi, :],
                    in0=pv[:, lo:hi, : DIM // 2],
                    in1=pv[:, lo:hi, DIM // 2 :],
                )
                nc.vector.tensor_reduce(
                    out=pred_halves[k // KH][:, kk * T + lo : kk * T + hi],
                    in_=fv[:, lo:hi, :],
                    axis=mybir.AxisListType.X,
                    op=mybir.AluOpType.add,
                )

            for k in range(NSPLIT):
                _piece(k, 0, TSPL)
            for k in range(NSPLIT):
                _piece(k, TSPL, T)
            for k in range(NSPLIT, K):
                _piece(k, 0, T)

            # --- epilogue: wm * (softplus(pred) - pred*label), sum over K -
            # softplus composed as relu(x) + ln(1 + exp(-|x|)) (no softplus
            # ACT table in this build).  Done per k-half so the first half
            # overlaps the second half's reduces.
            for hh in range(2):
                pred = pred_halves[hh]
                wm = wml_sb[:, hh * KH * T : (hh + 1) * KH * T]
                lab = wml_sb[:, (K + hh * KH) * T : (K + (hh + 1) * KH) * T]
                sp_a = epip.tile([P, KH * T], F32, tag=f"spa{hh}", name=f"spa{hh}")
                nc.scalar.activation(
                    out=sp_a[:], in_=pred[:], func=mybir.ActivationFunctionType.Abs
                )
                nc.scalar.activation(
                    out=sp_a[:], in_=sp_a[:],
                    func=mybir.ActivationFunctionType.Exp, scale=-1.0,
                )
                nc.scalar.activation(
                    out=sp_a[:], in_=sp_a[:],
                    func=mybir.ActivationFunctionType.Ln, bias=1.0,
                )
                sp_r = epip.tile([P, KH * T], F32, tag=f"spr{hh}", name=f"spr{hh}")
                nc.scalar.activation(
                    out=sp_r[:], in_=pred[:], func=mybir.ActivationFunctionType.Relu
                )
                t1 = epip.tile([P, KH * T], F32, tag=f"t1{hh}", name=f"t1{hh}")
                nc.vector.tensor_mul(out=t1[:], in0=pred[:], in1=lab)
                nc.vector.tensor_sub(out=sp_r[:], in0=sp_r[:], in1=t1[:])
                nc.vector.tensor_add(out=sp_r[:], in0=sp_r[:], in1=sp_a[:])
                nc.vector.tensor_mul(out=sp_r[:], in0=sp_r[:], in1=wm)
                nh = epip.tile([P, T], F32, tag=f"nh{hh}", name=f"nh{hh}")
                nc.vector.tensor_reduce(
                    out=nh[:],
                    in_=sp_r[:].rearrange("p (k t) -> p t k", k=KH),
                    axis=mybir.AxisListType.X,
                    op=mybir.AluOpType.add,
                )
                # each half's output DMA overlaps the other half's epilogue
                nc.sync.dma_start(out=out[:, hh * T : (hh + 1) * T], in_=nh[:])

    _split_multiwait(nc)
    lower_extended_insts(nc)

    # Hoist the library reload to the very front of the main block: the
    # ~10 us Q7 ucode load then overlaps the Bass preamble (sem init +
    # all-core start barrier) instead of serializing after it.  The reload
    # has no register or semaphore operands, and the const MEMSETs are
    # native Pool ops (not library ucode), so reordering is safe.
    mainb = nc.m.functions[0].blocks[0]
    il = mainb.instructions
    reloads = [i for i in il if "Reload" in type(i).__name__
               or getattr(i, "op_name", "") == "PseudoReloadLibraryIndex"]
    for r in reloads:
        il.remove(r)
    for pos, r in enumerate(reloads):
        il.insert(pos, r)
    _cached_nc = nc
    return nc


def _wrap(flat):
    """[n] int16 (flat[i] gathers to out slot (i%128, i//128)) -> the
    dma_gather idx tile layout: [16, n//16] with (p, s) = flat[s*16+p],
    replicated to 128 partitions."""
    n = flat.shape[0]
    return np.tile(flat.reshape(n // 16, 16).T, (8, 1))


def _prep_side(vals, emb, m_levels, tab_rows, dense_dt=TAB_DT):
    """Shared host prep for one side (ctx or neg) of one core.

    vals: [BL, J] int64 vocab ids.  Returns (table, denseA, idx_np, order)
    where order is the per-example column permutation applied (singles
    first), so callers can permute slot-aligned payloads identically.
    """
    J = vals.shape[1]
    u, inv = np.unique(vals.ravel(), return_inverse=True)
    ids = inv.reshape(BL, J)
    cnt = np.bincount(inv)
    singles = cnt[ids] == 1
    order = np.argsort(~singles, axis=1, kind="stable")
    sids = np.take_along_axis(ids, order, axis=1)
    tab = np.zeros((tab_rows, DIM), dtype=TAB_DT)
    tab[: len(u)] = emb[u].astype(TAB_DT)
    # dense streams packed p-major per level: row r <-> slot (p=r//mt, t=r%mt)
    # fully-dense levels go to the fp8 stream, mixed levels to the bf16 one.
    segs = []
    for j in range(J):
        m = m_levels[j]
        segs.append(
            tab[sids[:m, j].reshape(m // P, P).T.ravel()].astype(dense_dt)
        )
    denseA = np.concatenate(segs, axis=0)
    gsegs = [
        _wrap(sids[m_levels[j] :, j].astype(np.int16))
        for j in range(J)
        if m_levels[j] < BL
    ]
    idx_np = np.concatenate(gsegs, axis=1)
    return tab, denseA, idx_np, order


def kernel(contexts, focus_word, weight_mask, labels, ctx_emb, neg_emb):
    contexts = np.asarray(contexts)
    focus_word = np.asarray(focus_word)
    weight_mask = np.asarray(weight_mask, dtype=np.float32)
    labels = np.asarray(labels, dtype=np.float32)
    ctx_emb = np.asarray(ctx_emb, dtype=np.float32)
    neg_emb = np.asarray(neg_emb, dtype=np.float32)

    nc = _build()

    in_maps = []
    dens = []
    for i in range(NCORES):
        sl = slice(i * BL, (i + 1) * BL)
        ctx_i = contexts[sl].astype(np.int64)  # [BL, C]
        foc_i = focus_word[sl].astype(np.int64)  # [BL, K]
        wm_i = weight_mask[sl]  # [BL, K]
        lab_i = labels[sl]

        ctab, ctxA8_np, ctx_idx_np, _ = _prep_side(
            ctx_i, ctx_emb, M_CTX, CTX_ROWS, dense_dt=TAB8_DT
        )
        ntab, negA_np, neg_idx_np, n_order = _prep_side(
            foc_i, neg_emb, M_NEG, NEG_ROWS
        )
        # negative slots were permuted per-example: permute wm/labels too
        wm_s = np.take_along_axis(wm_i, n_order, axis=1)
        lab_s = np.take_along_axis(lab_i, n_order, axis=1)

        # wm/lab to [P, K*T]: (p, k*T+t) = value[e = t*128+p, k]
        wm_r = wm_s.reshape(T, P, K).transpose(1, 2, 0).reshape(P, K * T)
        lab_r = lab_s.reshape(T, P, K).transpose(1, 2, 0).reshape(P, K * T)
        wml_np = np.concatenate([wm_r, lab_r], axis=1)

        in_maps.append(
            {
                "ctx_tab": ctab,
                "neg_tab": ntab,
                "ctxA8": np.ascontiguousarray(ctxA8_np),
                "negA": np.ascontiguousarray(negA_np),
                "gidx": np.ascontiguousarray(
                    np.concatenate([ctx_idx_np, neg_idx_np], axis=1)
                ),
                "ident": np.eye(P, dtype=TAB_DT),
                "ident8": np.eye(P, dtype=TAB8_DT),
                "wml": np.ascontiguousarray(wml_np),
            }
        )
        dens.append(wm_i.sum(axis=1))  # [BL] row denominators

    res = run_bass_kernel_spmd(nc, in_maps, core_ids=list(range(NCORES)))

    total = 0.0
    for i in range(NCORES):
        o = res.results[i]["out"]  # [P, 2T]: two K-half numerators
        num = o[:, :T] + o[:, T:]
        num_e = num.T.reshape(BL)  # [BL] in example order
        total += float((num_e.astype(np.float64) / dens[i].astype(np.float64)).sum())
    return np.float32(total / B)


# revision 63
# speedup vs baseline: 1.0744x; 1.0148x over previous
"""CBOW forward (embedding lookup + pooled dot + weighted BCE) on 8 TRN2 cores.

Strategy: data-parallel over the batch; each core owns BL = 2048 examples.

The original kernel was bound by SWDGE descriptor generation: every gathered
row costs ~2.4 ns of serial GPSIMD (Pool engine) ucode time, and 36864
rows/core -> ~89 us.  This version removes ~80% of that by exploiting a
layout freedom: within an example, context positions (and negative slots,
together with their weight/label) are interchangeable.  Host-side we order
each example's rows so that rows which can be served by a *statically
shaped* dense stream come first:

  - For position/slot index j, a static "staircase" m_j says examples
    0..m_j-1 get row j from a dense stream tensor (packed host-side in
    exactly slot order), and examples >= m_j get it via dma_gather.
  - Each example's single-occurrence rows are placed in the dense slots
    first, so the dense stream is (nearly) duplication-free: it is a
    permutation of unique table rows, plus a few % duplicates where an
    example has fewer single-occurrence rows than dense slots.

Dense streams ride plain static DMAs on the two HWDGE queues (SP/ACT
engines), packed p-major host-side so each descriptor covers a 16-row
4 KB span (HWDGE charges ~1 ns/descriptor to the issuing engine).  Only
~6.5k rows/core (the staircase tails) still use dma_gather.

Context pooling runs on the otherwise-idle TensorEngine: identity-weight
matmuls accumulate the 10 position tiles into PSUM (f32), freeing the DVE
for the negative phase (mul + fold + reduce) and epilogue.

Tables are bf16; reduction and epilogue stay f32.
Host: per_row = num / sum_k(weight_mask); answer = mean over all rows.
"""

import numpy as np
import ml_dtypes

# run_bass_kernel_spmd under axon imports antenv.axon_hooks unconditionally;
# provide an in-process stub if the container image lacks that module.
import sys as _sys
import types as _types

try:
    import antenv.axon_hooks  # noqa: F401
except Exception:
    import antenv as _antenv

    _m = _types.ModuleType("antenv.axon_hooks")
    _m._hook = None
    _m.set_axon_ntff_profile_hook = lambda h: setattr(_m, "_hook", h)
    _m.get_axon_ntff_profile_hook = lambda: _m._hook
    _sys.modules["antenv.axon_hooks"] = _m
    _antenv.axon_hooks = _m

import concourse.bass as bass
from concourse import mybir
from concourse.bass_utils import run_bass_kernel_spmd
from concourse.tile import TileContext
from concourse.library_config import mlp as mlp_lib
from concourse.library_overlay import lower_extended_insts
# ---------------------------------------------------------------------------
# Workarounds for this walrus build (see notes below), self-contained.
# ---------------------------------------------------------------------------


def _split_multiwait(nc):
    """This walrus build rejects >1 sync-wait per instruction ("Too many sync
    wait commands").  Hoist extra SyncWaits onto NoOps inserted immediately
    before the instruction on the same engine (sequencer executes them in
    order, so cumulative wait semantics are unchanged)."""
    uid = 0
    for f in nc.m.functions:
        for b in f.blocks:
            il = b.instructions
            i = 0
            while i < len(il):
                inst = il[i]
                si = inst.sync_info
                if si is not None and si.on_wait and len(si.on_wait) > 1:
                    waits = list(si.on_wait)
                    si.on_wait = waits[-1:]
                    for w in waits[:-1]:
                        uid += 1
                        nop = mybir.InstNoOp(name=f"I-mwsplit-{uid}", ins=[], outs=[])
                        nop.engine = inst.engine
                        nop.sync_info = mybir.SyncInfo(on_wait=[w], on_update=[])
                        il.insert(i, nop)
                        i += 1
                i += 1


def _enable_dynamic_dma():
    """Without --dge-levels this walrus build logs "DynamicDMA is disabled"
    and silently compiles dynamic-AP DMAs as plain sequential copies."""
    from concourse import bass_utils as _bu

    if getattr(_bu.get_walrus_args, "_dyndma_patched", False):
        return
    _orig = _bu.get_walrus_args

    def _patched(arch, tmpdir, *, dve_root=None):
        return _orig(arch, tmpdir, dve_root=dve_root) + [
            "--dge-levels=vector_dynamic_offsets,scalar_dynamic_offset,dst_reduce"
        ]

    _patched._dyndma_patched = True
    _bu.get_walrus_args = _patched


_enable_dynamic_dma()


def _light_drain_and_barrier(self, tick_clock, wait_clock):
    """Tile teardown with sem-only engine barriers (saves ~2 us vs the
    full drain+barrier pair; waits split to 1/instruction for this walrus)."""
    from concourse.vector_clock import ScopedClock as _SC

    nc = self.nc
    probe = nc.sync.nop()
    wait_clock.add_sem_waits(probe.ins, _SC({None: tick_clock.global_clock}))
    si = probe.ins.sync_info
    waits = list(si.on_wait) if si is not None and si.on_wait else []
    if len(waits) > 1:
        si.on_wait = waits[:1]
        for w in waits[1:]:
            extra = nc.sync.nop()
            extra.ins.sync_info = mybir.SyncInfo(on_wait=[w], on_update=[])
    nc.sync.drain()
    nc.all_engine_barrier(sem_only=True)
    popped = nc._tile_sem_poison_stack.pop()
    assert popped is self._sem_poison
    nc.clear_and_free_semaphores(list(self.sems.allocated().values()))
    nc.all_engine_barrier(sem_only=True)


TileContext._drain_and_barrier = _light_drain_and_barrier

# ---------------------------------------------------------------------------
# Problem constants (hardcoded per the task spec).
# ---------------------------------------------------------------------------

B, C, K, DIM, VOCAB = 16384, 10, 8, 128, 100000
NCORES = 8
BL = B // NCORES  # 2048 examples per core
P = 128
T = BL // P  # 16 example slots per partition
CTX_ROWS = BL * C  # unique-ctx table rows (zero-padded), < 2^15
NEG_ROWS = BL * K  # unique-neg table rows (zero-padded)
NQ = 4  # SWDGE queues
F32 = mybir.dt.float32
I16 = mybir.dt.int16
EMB = mybir.dt.bfloat16
EMB8 = mybir.dt.float8e4
TAB_DT = ml_dtypes.bfloat16
TAB8_DT = mybir.dt.np(EMB8)

# Dense-stream staircases: m_j = number of examples whose j-th (sorted)
# row comes from the dense stream.  Multiples of 128.  Biased dense-heavy
# (beyond the single-occurrence means: ctx Bin(10,.815)->8.15, neg
# Bin(8,.849)->6.79): the extra dense slots are dup-filled (~2% more HBM
# bytes) but shrink the serial GPSIMD gather tail that gates the PSUM
# pooling and with it the whole DVE back half.
M_CTX = (2048,) * 6 + (1920, 1664, 1536, 1536)
M_NEG = (2048,) * 5 + (1792, 1536, 1536)
# Fully-dense levels (m == BL) stream in fp8-e4m3: the ctx tiles feed the
# PE pooling matmul directly (fp8 is a native matmul operand dtype); the
# neg tiles are upcast to bf16 on the idle Scalar engine before the DVE
# muls.  Mixed dense+gather levels stay bf16 (dma_gather rows must be a
# multiple of 256B, and one tile has one dtype).  Simulated end-to-end
# rel err of fp8 dense on both sides: ~8.6e-4 vs the 2e-2 gate.
CTX_N8 = sum(1 for m in M_CTX if m == BL)  # 6 fully-dense ctx levels
CTXA8_ROWS = sum(M_CTX)  # 18944: ALL ctx dense rows stream fp8
# Neg tiles stay bf16: a mixed bf16 x fp8 DVE mul measured at HALF the
# bf16 x bf16 rate, costing more than the fp8 DMA bytes saved.
NEGA_ROWS = sum(M_NEG)  # 15104 bf16 neg dense rows
CTX_G = [BL - m for m in M_CTX]  # per-position gather counts
NEG_G = [BL - m for m in M_NEG]
CTX_GCOLS = sum(g // 16 for g in CTX_G)  # 248 idx cols
NEG_GCOLS = sum(g // 16 for g in NEG_G)  # 160 idx cols
GCOLS = CTX_GCOLS + NEG_GCOLS
KH = K // 2
QW = 512  # PSUM bank width in f32: pooling/mul run in 4 column quarters

_cached_nc = None


def _build():
    global _cached_nc
    if _cached_nc is not None:
        return _cached_nc
    _orig_aeb = bass.Bass.all_engine_barrier

    def _semonly_aeb(self, *, sem_only=False):
        return _orig_aeb(self, sem_only=True)

    bass.Bass.all_engine_barrier = _semonly_aeb
    try:
        nc = bass.Bass(num_swdge_queues=NQ)
    finally:
        bass.Bass.all_engine_barrier = _orig_aeb

    ctx_tab = nc.declare_dram_parameter("ctx_tab", [CTX_ROWS, DIM], EMB, isOutput=False)
    neg_tab = nc.declare_dram_parameter("neg_tab", [NEG_ROWS, DIM], EMB, isOutput=False)
    ctxA8 = nc.declare_dram_parameter("ctxA8", [CTXA8_ROWS, DIM], EMB8, isOutput=False)
    negA = nc.declare_dram_parameter("negA", [NEGA_ROWS, DIM], EMB, isOutput=False)
    gidx = nc.declare_dram_parameter("gidx", [P, GCOLS], I16, isOutput=False)
    ident_d = nc.declare_dram_parameter("ident", [P, P], EMB, isOutput=False)
    ident8_d = nc.declare_dram_parameter("ident8", [P, P], EMB8, isOutput=False)
    # wm cols [0, K*T), labels cols [K*T, 2*K*T)
    wml = nc.declare_dram_parameter("wml", [P, 2 * K * T], F32, isOutput=False)
    out = nc.declare_dram_parameter("out", [P, 2 * T], F32, isOutput=True)

    # Issue the library reload in the main block, before the Tile preamble:
    # the Q7 ucode load (~11 us) then overlaps the EVSEM startup barriers.
    nc.gpsimd.load_library(mlp_lib)

    with TileContext(nc) as tc:
        with (
            tc.tile_pool(name="idxp", bufs=1) as idxp,
            tc.tile_pool(name="tiles", bufs=1) as tiles,
            tc.psum_pool(name="ps", bufs=1) as psp,
            tc.tile_pool(name="prod", bufs=3) as prodp,
            tc.tile_pool(name="fold", bufs=3) as foldp,
            tc.tile_pool(name="epi", bufs=1) as epip,
        ):
            # --- idx / wml loads (SP queue, small) -----------------------
            gidx_sb = idxp.tile([P, GCOLS], I16, tag="gidx", name="gidx")
            wml_sb = idxp.tile([P, 2 * K * T], F32)

            # --- position / k tiles --------------------------------------
            # ALL ctx dense data is fp8 (it only feeds the PE pooling
            # matmul).  For staircase levels (c>=CTX_N8) the dense prefix
            # and the gathered tail live in SEPARATE tiles: the gather tail
            # is bf16 (dma_gather rows must be 256B-aligned), and the split
            # means the PSUM chains for quarters 0-2 never depend on any
            # gather — only the q3 chain touches the bf16 gather tiles, via
            # column-subrange matmuls.
            ctx_t = [
                tiles.tile([P, BL], EMB8, tag=f"ct{c}", name=f"ct{c}")
                for c in range(CTX_N8)
            ]
            ctxd_t, ctxg_t = {}, {}
            for c in range(CTX_N8, C):
                mt = M_CTX[c] // P
                ctxd_t[c] = tiles.tile(
                    [P, mt * DIM], EMB8, tag=f"cd{c}", name=f"cd{c}"
                )
                ctxg_t[c] = tiles.tile(
                    [P, (T - mt) * DIM], EMB, tag=f"cg{c}", name=f"cg{c}"
                )
            neg_t = [
                tiles.tile([P, BL], EMB, tag=f"nt{k}", name=f"nt{k}") for k in range(K)
            ]

            # identity weights for the PE pooling copies come from DRAM so
            # the GPSIMD stream stays free for the gathers
            ident = idxp.tile([P, P], EMB, tag="ident", name="ident")
            ident8 = idxp.tile([P, P], EMB8, tag="ident8", name="ident8")
            # Queue order matters: the ctx tiles gate the PE chains and with
            # them the whole DVE back half, so they go FIRST on both HWDGE
            # queues, after only the tiny weights/idx tiles each consumer
            # needs early (ident8 for the PE, gidx for the gathers).  wml is
            # only read by the epilogue and loads last.
            nc.sync.dma_start(out=ident8[:], in_=ident8_d[:])
            nc.scalar.dma_start(out=ident[:], in_=ident_d[:])

            # --- dense streams (HWDGE queues) ---------------------------
            # A is packed p-major host-side: row r of level j holds the row
            # for slot (p = r // mt, t = r % mt), so each descriptor moves a
            # contiguous mt-row span into one partition.  c0/c1 go ahead of
            # even gidx: the PE chains start on c0's arrival, while the
            # gathers (behind the ~15us GPSIMD library load) don't need
            # gidx until later.  wml is only read by the epilogue and loads
            # last.
            offs = [sum(M_CTX[:c]) for c in range(C)]
            for c in range(C):
                m = M_CTX[c]
                dst_tile = ctx_t[c] if c < CTX_N8 else ctxd_t[c]
                eng = nc.sync if c % 2 == 0 else nc.scalar
                eng.dma_start(
                    out=dst_tile[:].rearrange("p (t d) -> p t d", d=DIM),
                    in_=ctxA8[offs[c] : offs[c] + m, :].rearrange(
                        "(p t) d -> p t d", p=P
                    ),
                )
                if c == 1:
                    nc.sync.dma_start(out=gidx_sb[:], in_=gidx[:])
            noffs = [sum(M_NEG[:k]) for k in range(K)]
            for k in range(2):
                m = M_NEG[k]
                eng = nc.sync if k % 2 == 0 else nc.scalar
                eng.dma_start(
                    out=neg_t[k][:].rearrange("p (t d) -> p t d", d=DIM)[
                        :, : m // P, :
                    ],
                    in_=negA[noffs[k] : noffs[k] + m, :].rearrange(
                        "(p t) d -> p t d", p=P
                    ),
                )
            nc.scalar.dma_start(out=wml_sb[:], in_=wml[:])

            # --- gathers next: they serialize on GPSIMD.  Ctx first (they
            # gate the PSUM pooling and with it the whole DVE back half),
            # largest first within each side; queue round-robin.
            regs = {}
            for n in sorted({*CTX_G, *NEG_G} - {0}):
                regs[n] = nc.gpsimd.to_reg(n)
            idx_off = {}
            off = 0
            for c in range(C):
                if CTX_G[c]:
                    idx_off[("c", c)] = off
                    off += CTX_G[c] // 16
            for k in range(K):
                if NEG_G[k]:
                    idx_off[("n", k)] = off
                    off += NEG_G[k] // 16
            order = sorted(
                idx_off,
                key=lambda s: (
                    s[0] != "c",
                    -(CTX_G[s[1]] if s[0] == "c" else NEG_G[s[1]]),
                ),
            )
            # queues 1-3 only: queue 0 carries the Pool-static neg dense
            # streams, whose packets would contend with the gather drain
            for qn, key in enumerate(order):
                side, j = key
                g = CTX_G[j] if side == "c" else NEG_G[j]
                if side == "c":
                    out_ap = ctxg_t[j][:].rearrange("p (t d) -> p t d", d=DIM)
                else:
                    m = M_NEG[j]
                    out_ap = neg_t[j][:].rearrange("p (t d) -> p t d", d=DIM)[
                        :, m // P :, :
                    ]
                tab = ctx_tab if side == "c" else neg_tab
                o = idx_off[key]
                nc.gpsimd.dma_gather(
                    out_ap,
                    tab[:],
                    gidx_sb[:, o : o + g // 16],
                    g, regs[g], DIM,
                    single_packet=False,
                    queue_num=1 + qn % (NQ - 1),
                )

            # --- neg k2-7 as SWDGE static DMAs behind the gathers --------
            # (their ~2.9MB stays off the HWDGE queues that must deliver
            # the ctx tiles early; the DVE consumes them late)
            for k in range(2, K):
                m = M_NEG[k]
                nc.gpsimd.dma_start(
                    out=neg_t[k][:].rearrange("p (t d) -> p t d", d=DIM)[
                        :, : m // P, :
                    ],
                    in_=negA[noffs[k] : noffs[k] + m, :].rearrange(
                        "(p t) d -> p t d", p=P
                    ),
                )

            # --- ctx pooling on the PE: identity matmuls accumulate the 10
            # position tiles into PSUM f32, one 512-col quarter per bank.
            acc_ps = [
                psp.tile([P, QW], F32, tag=f"acc{q}", name=f"acc{q}")
                for q in range(BL // QW)
            ]
            # quarters 0-2 read only fp8 dense tiles (no gather deps); the
            # q3 chain (cols [1536, 2048)) mixes per-c dense/gather column
            # subranges — every column is first written by c0's start=True
            # full-width matmul, so later subrange matmuls all accumulate.
            NQ3 = (BL // QW) - 1
            for q in range(NQ3):
                for c in range(C):
                    rhs_tile = ctx_t[c] if c < CTX_N8 else ctxd_t[c]
                    nc.tensor.matmul(
                        out=acc_ps[q][:],
                        lhsT=ident8[:],
                        rhs=rhs_tile[:, q * QW : (q + 1) * QW],
                        start=(c == 0),
                        stop=(c == C - 1),
                    )
            q3_parts = []  # (rhs_ap, out_lo, out_hi, fp8)
            for c in range(C):
                if c < CTX_N8:
                    q3_parts.append((ctx_t[c][:, NQ3 * QW :], 0, QW, True))
                else:
                    mt = M_CTX[c] // P
                    dcols = (mt - 12) * DIM  # dense cols within q3
                    if dcols > 0:
                        q3_parts.append(
                            (ctxd_t[c][:, 12 * DIM :], 0, dcols, True)
                        )
                    q3_parts.append((ctxg_t[c][:], dcols, QW, False))
            for i, (rhs, lo, hi, fp8) in enumerate(q3_parts):
                nc.tensor.matmul(
                    out=acc_ps[NQ3][:, lo:hi],
                    lhsT=(ident8 if fp8 else ident)[:],
                    rhs=rhs,
                    start=(i == 0),
                    stop=(i == len(q3_parts) - 1),
                )

            # --- PSUM -> SBUF src_acc on the (idle) Scalar engine: DVE reads
            # PSUM at half rate, so the muls read an SBUF copy instead.
            src_acc = idxp.tile([P, BL], EMB, tag="srcacc", name="srcacc")
            for q in range(BL // QW):
                nc.scalar.activation(
                    out=src_acc[:, q * QW : (q + 1) * QW],
                    in_=acc_ps[q][:],
                    func=mybir.ActivationFunctionType.Copy,
                )

            # --- negatives: mul + fold + reduce on the DVE ----------------
            pred_halves = [
                epip.tile([P, KH * T], F32, tag="predlo", name="predlo"),
                epip.tile([P, KH * T], F32, tag="predhi", name="predhi"),
            ]
            # k0/k1 run in a 3/4 + 1/4 split: slots t<12 touch no gathered
            # data, so the DVE starts on them while the q3 (t>=12) gathers
            # and their PSUM chain are still in flight.  Remaining k's run
            # full-width (the per-op ~385ns DVE fixed cost makes finer
            # splits a net loss).
            TSPL = 12
            NSPLIT = 3  # how many leading k's run split
            prods, folds = {}, {}
            for k in range(K):
                prods[k] = prodp.tile([P, BL], EMB, tag="prodch", name=f"prod{k}")
                folds[k] = foldp.tile([P, BL // 2], EMB, tag="foldch", name=f"fold{k}")

            def _piece(k, lo, hi):
                kk = k % KH
                pv = prods[k][:].rearrange("p (t d) -> p t d", d=DIM)
                fv = folds[k][:].rearrange("p (t d) -> p t d", d=DIM // 2)
                nc.vector.tensor_mul(
                    out=prods[k][:, lo * DIM : hi * DIM],
                    in0=src_acc[:, lo * DIM : hi * DIM],
                    in1=neg_t[k][:, lo * DIM : hi * DIM],
                )
                nc.vector.tensor_add(
                    out=fv[:, lo:hi, :],
                    in0=pv[:, lo:hi, : DIM // 2],
                    in1=pv[:, lo:hi, DIM // 2 :],
                )
                nc.vector.tensor_reduce(
                    out=pred_halves[k // KH][:, kk * T + lo : kk * T + hi],
                    in_=fv[:, lo:hi, :],
                    axis=mybir.AxisListType.X,
                    op=mybir.AluOpType.add,
                )

            for k in range(NSPLIT):
                _piece(k, 0, TSPL)
            for k in range(NSPLIT):
                _piece(k, TSPL, T)
            for k in range(NSPLIT, K):
                _piece(k, 0, T)

            # --- epilogue: wm * (softplus(pred) - pred*label), sum over K -
            # softplus composed as relu(x) + ln(1 + exp(-|x|)) (no softplus
            # ACT table in this build).  Done per k-half so the first half
            # overlaps the second half's reduces.
            for hh in range(2):
                pred = pred_halves[hh]
                wm = wml_sb[:, hh * KH * T : (hh + 1) * KH * T]
                lab = wml_sb[:, (K + hh * KH) * T : (K + (hh + 1) * KH) * T]
                sp_a = epip.tile([P, KH * T], F32, tag=f"spa{hh}", name=f"spa{hh}")
                nc.scalar.activation(
                    out=sp_a[:], in_=pred[:], func=mybir.ActivationFunctionType.Abs
                )
                nc.scalar.activation(
                    out=sp_a[:], in_=sp_a[:],
                    func=mybir.ActivationFunctionType.Exp, scale=-1.0,
                )
                nc.scalar.activation(
                    out=sp_a[:], in_=sp_a[:],
                    func=mybir.ActivationFunctionType.Ln, bias=1.0,
                )
                sp_r = epip.tile([P, KH * T], F32, tag=f"spr{hh}", name=f"spr{hh}")
                nc.scalar.activation(
                    out=sp_r[:], in_=pred[:], func=mybir.ActivationFunctionType.Relu
                )
                t1 = epip.tile([P, KH * T], F32, tag=f"t1{hh}", name=f"t1{hh}")
                nc.vector.tensor_mul(out=t1[:], in0=pred[:], in1=lab)
                nc.vector.tensor_sub(out=sp_r[:], in0=sp_r[:], in1=t1[:])
                nc.vector.tensor_add(out=sp_r[:], in0=sp_r[:], in1=sp_a[:])
                nc.vector.tensor_mul(out=sp_r[:], in0=sp_r[:], in1=wm)
                nh = epip.tile([P, T], F32, tag=f"nh{hh}", name=f"nh{hh}")
                nc.vector.tensor_reduce(
                    out=nh[:],
                    in_=sp_r[:].rearrange("p (k t) -> p t k", k=KH),
                    axis=mybir.AxisListType.X,
                    op=mybir.AluOpType.add,
                )
                # each half's output DMA overlaps the other half's epilogue
                nc.sync.dma_start(out=out[:, hh * T : (hh + 1) * T], in_=nh[:])

    _split_multiwait(nc)
    lower_extended_insts(nc)

    # Hoist the library reload to the very front of the main block: the
    # ~10 us Q7 ucode load then overlaps the Bass preamble (sem init +
    # all-core start barrier) instead of serializing after it.  The reload
    # has no register or semaphore operands, and the const MEMSETs are
    # native Pool ops (not library ucode), so reordering is safe.
    mainb = nc.m.functions[0].blocks[0]
    il = mainb.instructions
    reloads = [i for i in il if "Reload" in type(i).__name__
               or getattr(i, "op_name", "") == "PseudoReloadLibraryIndex"]
    for r in reloads:
        il.remove(r)
    for pos, r in enumerate(reloads):
        il.insert(pos, r)
    _cached_nc = nc
    return nc


def _wrap(flat):
    """[n] int16 (flat[i] gathers to out slot (i%128, i//128)) -> the
    dma_gather idx tile layout: [16, n//16] with (p, s) = flat[s*16+p],
    replicated to 128 partitions."""
    n = flat.shape[0]
    return np.tile(flat.reshape(n // 16, 16).T, (8, 1))


def _prep_side(vals, emb, m_levels, tab_rows, dense_dt=TAB_DT):
    """Shared host prep for one side (ctx or neg) of one core.

    vals: [BL, J] int64 vocab ids.  Returns (table, denseA, idx_np, order)
    where order is the per-example column permutation applied (singles
    first), so callers can permute slot-aligned payloads identically.
    """
    J = vals.shape[1]
    u, inv = np.unique(vals.ravel(), return_inverse=True)
    ids = inv.reshape(BL, J)
    cnt = np.bincount(inv)
    singles = cnt[ids] == 1
    order = np.argsort(~singles, axis=1, kind="stable")
    sids = np.take_along_axis(ids, order, axis=1)
    tab = np.zeros((tab_rows, DIM), dtype=TAB_DT)
    tab[: len(u)] = emb[u].astype(TAB_DT)
    # dense streams packed p-major per level: row r <-> slot (p=r//mt, t=r%mt)
    # fully-dense levels go to the fp8 stream, mixed levels to the bf16 one.
    segs = []
    for j in range(J):
        m = m_levels[j]
        segs.append(
            tab[sids[:m, j].reshape(m // P, P).T.ravel()].astype(dense_dt)
        )
    denseA = np.concatenate(segs, axis=0)
    gsegs = [
        _wrap(sids[m_levels[j] :, j].astype(np.int16))
        for j in range(J)
        if m_levels[j] < BL
    ]
    idx_np = np.concatenate(gsegs, axis=1)
    return tab, denseA, idx_np, order


def kernel(contexts, focus_word, weight_mask, labels, ctx_emb, neg_emb):
    contexts = np.asarray(contexts)
    focus_word = np.asarray(focus_word)
    weight_mask = np.asarray(weight_mask, dtype=np.float32)
    labels = np.asarray(labels, dtype=np.float32)
    ctx_emb = np.asarray(ctx_emb, dtype=np.float32)
    neg_emb = np.asarray(neg_emb, dtype=np.float32)

    nc = _build()

    in_maps = []
    dens = []
    for i in range(NCORES):
        sl = slice(i * BL, (i + 1) * BL)
        ctx_i = contexts[sl].astype(np.int64)  # [BL, C]
        foc_i = focus_word[sl].astype(np.int64)  # [BL, K]
        wm_i = weight_mask[sl]  # [BL, K]
        lab_i = labels[sl]

        ctab, ctxA8_np, ctx_idx_np, _ = _prep_side(
            ctx_i, ctx_emb, M_CTX, CTX_ROWS, dense_dt=TAB8_DT
        )
        ntab, negA_np, neg_idx_np, n_order = _prep_side(
            foc_i, neg_emb, M_NEG, NEG_ROWS
        )
        # negative slots were permuted per-example: permute wm/labels too
        wm_s = np.take_along_axis(wm_i, n_order, axis=1)
        lab_s = np.take_along_axis(lab_i, n_order, axis=1)

        # wm/lab to [P, K*T]: (p, k*T+t) = value[e = t*128+p, k]
        wm_r = wm_s.reshape(T, P, K).transpose(1, 2, 0).reshape(P, K * T)
        lab_r = lab_s.reshape(T, P, K).transpose(1, 2, 0).reshape(P, K * T)
        wml_np = np.concatenate([wm_r, lab_r], axis=1)

        in_maps.append(
            {
                "ctx_tab": ctab,
                "neg_tab": ntab,
                "ctxA8": np.ascontiguousarray(ctxA8_np),
                "negA": np.ascontiguousarray(negA_np),
                "gidx": np.ascontiguousarray(
                    np.concatenate([ctx_idx_np, neg_idx_np], axis=1)
                ),
                "ident": np.eye(P, dtype=TAB_DT),
                "ident8": np.eye(P, dtype=TAB8_DT),
                "wml": np.ascontiguousarray(wml_np),
            }
        )
        dens.append(wm_i.sum(axis=1))  # [BL] row denominators

    res = run_bass_kernel_spmd(nc, in_maps, core_ids=list(range(NCORES)))

    total = 0.0
    for i in range(NCORES):
        o = res.results[i]["out"]  # [P, 2T]: two K-half numerators
        num = o[:, :T] + o[:, T:]
        num_e = num.T.reshape(BL)  # [BL] in example order
        total += float((num_e.astype(np.float64) / dens[i].astype(np.float64)).sum())
    return np.float32(total / B)


# revision 65
# speedup vs baseline: 1.1270x; 1.0490x over previous
"""CBOW forward (embedding lookup + pooled dot + weighted BCE) on 8 TRN2 cores.

Strategy: data-parallel over the batch; each core owns BL = 2048 examples.

The original kernel was bound by SWDGE descriptor generation: every gathered
row costs ~2.4 ns of serial GPSIMD (Pool engine) ucode time, and 36864
rows/core -> ~89 us.  This version removes ~80% of that by exploiting a
layout freedom: within an example, context positions (and negative slots,
together with their weight/label) are interchangeable.  Host-side we order
each example's rows so that rows which can be served by a *statically
shaped* dense stream come first:

  - For position/slot index j, a static "staircase" m_j says examples
    0..m_j-1 get row j from a dense stream tensor (packed host-side in
    exactly slot order), and examples >= m_j get it via dma_gather.
  - Each example's single-occurrence rows are placed in the dense slots
    first, so the dense stream is (nearly) duplication-free: it is a
    permutation of unique table rows, plus a few % duplicates where an
    example has fewer single-occurrence rows than dense slots.

Dense streams ride plain static DMAs on the two HWDGE queues (SP/ACT
engines), packed p-major host-side so each descriptor covers a 16-row
4 KB span (HWDGE charges ~1 ns/descriptor to the issuing engine).  Only
~6.5k rows/core (the staircase tails) still use dma_gather.

Context pooling runs on the otherwise-idle TensorEngine: identity-weight
matmuls accumulate the 10 position tiles into PSUM (f32), freeing the DVE
for the negative phase (mul + fold + reduce) and epilogue.

Tables are bf16; reduction and epilogue stay f32.
Host: per_row = num / sum_k(weight_mask); answer = mean over all rows.
"""

import numpy as np
import ml_dtypes

# run_bass_kernel_spmd under axon imports antenv.axon_hooks unconditionally;
# provide an in-process stub if the container image lacks that module.
import sys as _sys
import types as _types

try:
    import antenv.axon_hooks  # noqa: F401
except Exception:
    import antenv as _antenv

    _m = _types.ModuleType("antenv.axon_hooks")
    _m._hook = None
    _m.set_axon_ntff_profile_hook = lambda h: setattr(_m, "_hook", h)
    _m.get_axon_ntff_profile_hook = lambda: _m._hook
    _sys.modules["antenv.axon_hooks"] = _m
    _antenv.axon_hooks = _m

import concourse.bass as bass
from concourse import mybir
from concourse.bass_utils import run_bass_kernel_spmd
from concourse.tile import TileContext
from concourse.library_config import mlp as mlp_lib
from concourse.library_overlay import lower_extended_insts
# ---------------------------------------------------------------------------
# Workarounds for this walrus build (see notes below), self-contained.
# ---------------------------------------------------------------------------


def _split_multiwait(nc):
    """This walrus build rejects >1 sync-wait per instruction ("Too many sync
    wait commands").  Hoist extra SyncWaits onto NoOps inserted immediately
    before the instruction on the same engine (sequencer executes them in
    order, so cumulative wait semantics are unchanged)."""
    uid = 0
    for f in nc.m.functions:
        for b in f.blocks:
            il = b.instructions
            i = 0
            while i < len(il):
                inst = il[i]
                si = inst.sync_info
                if si is not None and si.on_wait and len(si.on_wait) > 1:
                    waits = list(si.on_wait)
                    si.on_wait = waits[-1:]
                    for w in waits[:-1]:
                        uid += 1
                        nop = mybir.InstNoOp(name=f"I-mwsplit-{uid}", ins=[], outs=[])
                        nop.engine = inst.engine
                        nop.sync_info = mybir.SyncInfo(on_wait=[w], on_update=[])
                        il.insert(i, nop)
                        i += 1
                i += 1


def _enable_dynamic_dma():
    """Without --dge-levels this walrus build logs "DynamicDMA is disabled"
    and silently compiles dynamic-AP DMAs as plain sequential copies."""
    from concourse import bass_utils as _bu

    if getattr(_bu.get_walrus_args, "_dyndma_patched", False):
        return
    _orig = _bu.get_walrus_args

    def _patched(arch, tmpdir, *, dve_root=None):
        return _orig(arch, tmpdir, dve_root=dve_root) + [
            "--dge-levels=vector_dynamic_offsets,scalar_dynamic_offset,dst_reduce"
        ]

    _patched._dyndma_patched = True
    _bu.get_walrus_args = _patched


_enable_dynamic_dma()


def _light_drain_and_barrier(self, tick_clock, wait_clock):
    """Tile teardown with sem-only engine barriers (saves ~2 us vs the
    full drain+barrier pair; waits split to 1/instruction for this walrus)."""
    from concourse.vector_clock import ScopedClock as _SC

    nc = self.nc
    probe = nc.sync.nop()
    wait_clock.add_sem_waits(probe.ins, _SC({None: tick_clock.global_clock}))
    si = probe.ins.sync_info
    waits = list(si.on_wait) if si is not None and si.on_wait else []
    if len(waits) > 1:
        si.on_wait = waits[:1]
        for w in waits[1:]:
            extra = nc.sync.nop()
            extra.ins.sync_info = mybir.SyncInfo(on_wait=[w], on_update=[])
    nc.sync.drain()
    nc.all_engine_barrier(sem_only=True)
    popped = nc._tile_sem_poison_stack.pop()
    assert popped is self._sem_poison
    nc.clear_and_free_semaphores(list(self.sems.allocated().values()))
    nc.all_engine_barrier(sem_only=True)


TileContext._drain_and_barrier = _light_drain_and_barrier

# ---------------------------------------------------------------------------
# Problem constants (hardcoded per the task spec).
# ---------------------------------------------------------------------------

B, C, K, DIM, VOCAB = 16384, 10, 8, 128, 100000
NCORES = 8
BL = B // NCORES  # 2048 examples per core
P = 128
T = BL // P  # 16 example slots per partition
CTX_ROWS = BL * C  # unique-ctx table rows (zero-padded), < 2^15
NEG_ROWS = BL * K  # unique-neg table rows (zero-padded)
NQ = 4  # SWDGE queues
F32 = mybir.dt.float32
I16 = mybir.dt.int16
EMB = mybir.dt.bfloat16
EMB8 = mybir.dt.float8e4
TAB_DT = ml_dtypes.bfloat16
TAB8_DT = mybir.dt.np(EMB8)

# Dense-stream staircases: m_j = number of examples whose j-th (sorted)
# row comes from the dense stream.  Multiples of 128.  Biased dense-heavy
# (beyond the single-occurrence means: ctx Bin(10,.815)->8.15, neg
# Bin(8,.849)->6.79): the extra dense slots are dup-filled (~2% more HBM
# bytes) but shrink the serial GPSIMD gather tail that gates the PSUM
# pooling and with it the whole DVE back half.
M_CTX = (2048,) * 6 + (1920, 1664, 1536, 1536)
M_NEG = (2048,) * 5 + (1792, 1536, 1536)
# Fully-dense levels (m == BL) stream in fp8-e4m3: the ctx tiles feed the
# PE pooling matmul directly (fp8 is a native matmul operand dtype); the
# neg tiles are upcast to bf16 on the idle Scalar engine before the DVE
# muls.  Mixed dense+gather levels stay bf16 (dma_gather rows must be a
# multiple of 256B, and one tile has one dtype).  Simulated end-to-end
# rel err of fp8 dense on both sides: ~8.6e-4 vs the 2e-2 gate.
CTX_N8 = sum(1 for m in M_CTX if m == BL)  # 6 fully-dense ctx levels
CTXA8_ROWS = sum(M_CTX)  # 18944: ALL ctx dense rows stream fp8
# Neg tiles stay bf16: a mixed bf16 x fp8 DVE mul measured at HALF the
# bf16 x bf16 rate, costing more than the fp8 DMA bytes saved.
NEGA_ROWS = sum(M_NEG)  # 15104 bf16 neg dense rows
CTX_G = [BL - m for m in M_CTX]  # per-position gather counts
NEG_G = [BL - m for m in M_NEG]
CTX_GCOLS = sum(g // 16 for g in CTX_G)  # 248 idx cols
NEG_GCOLS = sum(g // 16 for g in NEG_G)  # 160 idx cols
GCOLS = CTX_GCOLS + NEG_GCOLS
KH = K // 2
QW = 512  # PSUM bank width in f32: pooling/mul run in 4 column quarters

_cached_nc = None


def _build():
    global _cached_nc
    if _cached_nc is not None:
        return _cached_nc
    _orig_aeb = bass.Bass.all_engine_barrier

    def _semonly_aeb(self, *, sem_only=False):
        return _orig_aeb(self, sem_only=True)

    bass.Bass.all_engine_barrier = _semonly_aeb
    try:
        nc = bass.Bass(num_swdge_queues=NQ)
    finally:
        bass.Bass.all_engine_barrier = _orig_aeb

    ctx_tab = nc.declare_dram_parameter("ctx_tab", [CTX_ROWS, DIM], EMB, isOutput=False)
    neg_tab = nc.declare_dram_parameter("neg_tab", [NEG_ROWS, DIM], EMB, isOutput=False)
    ctxA8 = nc.declare_dram_parameter("ctxA8", [CTXA8_ROWS, DIM], EMB8, isOutput=False)
    negA = nc.declare_dram_parameter("negA", [NEGA_ROWS, DIM], EMB, isOutput=False)
    gidx = nc.declare_dram_parameter("gidx", [P, GCOLS], I16, isOutput=False)
    ident_d = nc.declare_dram_parameter("ident", [P, P], EMB, isOutput=False)
    ident8_d = nc.declare_dram_parameter("ident8", [P, P], EMB8, isOutput=False)
    # wm cols [0, K*T), labels cols [K*T, 2*K*T)
    wml = nc.declare_dram_parameter("wml", [P, 2 * K * T], F32, isOutput=False)
    out = nc.declare_dram_parameter("out", [P, 2 * T], F32, isOutput=True)

    # Issue the library reload in the main block, before the Tile preamble:
    # the Q7 ucode load (~11 us) then overlaps the EVSEM startup barriers.
    nc.gpsimd.load_library(mlp_lib)

    with TileContext(nc) as tc:
        with (
            tc.tile_pool(name="idxp", bufs=1) as idxp,
            tc.tile_pool(name="tiles", bufs=1) as tiles,
            tc.psum_pool(name="ps", bufs=1) as psp,
            tc.tile_pool(name="prod", bufs=3) as prodp,
            tc.tile_pool(name="fold", bufs=3) as foldp,
            tc.tile_pool(name="epi", bufs=1) as epip,
        ):
            # --- idx / wml loads (SP queue, small) -----------------------
            gidx_sb = idxp.tile([P, GCOLS], I16, tag="gidx", name="gidx")
            wml_sb = idxp.tile([P, 2 * K * T], F32)

            # --- position / k tiles --------------------------------------
            # ALL ctx dense data is fp8 (it only feeds the PE pooling
            # matmul).  For staircase levels (c>=CTX_N8) the dense prefix
            # and the gathered tail live in SEPARATE tiles: the gather tail
            # is bf16 (dma_gather rows must be 256B-aligned), and the split
            # means the PSUM chains for quarters 0-2 never depend on any
            # gather — only the q3 chain touches the bf16 gather tiles, via
            # column-subrange matmuls.
            ctx_t = [
                tiles.tile([P, BL], EMB8, tag=f"ct{c}", name=f"ct{c}")
                for c in range(CTX_N8)
            ]
            ctxd_t, ctxg_t = {}, {}
            for c in range(CTX_N8, C):
                mt = M_CTX[c] // P
                ctxd_t[c] = tiles.tile(
                    [P, mt * DIM], EMB8, tag=f"cd{c}", name=f"cd{c}"
                )
                ctxg_t[c] = tiles.tile(
                    [P, (T - mt) * DIM], EMB, tag=f"cg{c}", name=f"cg{c}"
                )
            neg_t = [
                tiles.tile([P, BL], EMB, tag=f"nt{k}", name=f"nt{k}") for k in range(K)
            ]

            # identity weights for the PE pooling copies come from DRAM so
            # the GPSIMD stream stays free for the gathers
            ident = idxp.tile([P, P], EMB, tag="ident", name="ident")
            ident8 = idxp.tile([P, P], EMB8, tag="ident8", name="ident8")
            # Queue order matters: the ctx tiles gate the PE chains and with
            # them the whole DVE back half, so they go FIRST on both HWDGE
            # queues, after only the tiny weights/idx tiles each consumer
            # needs early (ident8 for the PE, gidx for the gathers).  wml is
            # only read by the epilogue and loads last.
            nc.sync.dma_start(out=ident8[:], in_=ident8_d[:])
            nc.scalar.dma_start(out=ident[:], in_=ident_d[:])

            # --- dense streams (HWDGE queues) ---------------------------
            # A is packed p-major host-side: row r of level j holds the row
            # for slot (p = r // mt, t = r % mt), so each descriptor moves a
            # contiguous mt-row span into one partition.  c0/c1 go ahead of
            # even gidx: the PE chains start on c0's arrival, while the
            # gathers (behind the ~15us GPSIMD library load) don't need
            # gidx until later.  wml is only read by the epilogue and loads
            # last.
            offs = [sum(M_CTX[:c]) for c in range(C)]
            for c in range(C):
                m = M_CTX[c]
                dst_tile = ctx_t[c] if c < CTX_N8 else ctxd_t[c]
                eng = nc.sync if c % 2 == 0 else nc.scalar
                eng.dma_start(
                    out=dst_tile[:].rearrange("p (t d) -> p t d", d=DIM),
                    in_=ctxA8[offs[c] : offs[c] + m, :].rearrange(
                        "(p t) d -> p t d", p=P
                    ),
                )
                if c == 1:
                    nc.sync.dma_start(out=gidx_sb[:], in_=gidx[:])
            noffs = [sum(M_NEG[:k]) for k in range(K)]
            for k in range(2):
                m = M_NEG[k]
                eng = nc.sync if k % 2 == 0 else nc.scalar
                eng.dma_start(
                    out=neg_t[k][:].rearrange("p (t d) -> p t d", d=DIM)[
                        :, : m // P, :
                    ],
                    in_=negA[noffs[k] : noffs[k] + m, :].rearrange(
                        "(p t) d -> p t d", p=P
                    ),
                )
            nc.scalar.dma_start(out=wml_sb[:], in_=wml[:])

            # --- gathers next: they serialize on GPSIMD.  Ctx first (they
            # gate the PSUM pooling and with it the whole DVE back half),
            # largest first within each side; queue round-robin.
            regs = {}
            for n in sorted({*CTX_G, *NEG_G} - {0}):
                regs[n] = nc.gpsimd.to_reg(n)
            idx_off = {}
            off = 0
            for c in range(C):
                if CTX_G[c]:
                    idx_off[("c", c)] = off
                    off += CTX_G[c] // 16
            for k in range(K):
                if NEG_G[k]:
                    idx_off[("n", k)] = off
                    off += NEG_G[k] // 16
            order = sorted(
                idx_off,
                key=lambda s: (
                    s[0] != "c",
                    -(CTX_G[s[1]] if s[0] == "c" else NEG_G[s[1]]),
                ),
            )
            # queues 1-3 only: queue 0 carries the Pool-static neg dense
            # streams, whose packets would contend with the gather drain
            for qn, key in enumerate(order):
                side, j = key
                g = CTX_G[j] if side == "c" else NEG_G[j]
                if side == "c":
                    out_ap = ctxg_t[j][:].rearrange("p (t d) -> p t d", d=DIM)
                else:
                    m = M_NEG[j]
                    out_ap = neg_t[j][:].rearrange("p (t d) -> p t d", d=DIM)[
                        :, m // P :, :
                    ]
                tab = ctx_tab if side == "c" else neg_tab
                o = idx_off[key]
                nc.gpsimd.dma_gather(
                    out_ap,
                    tab[:],
                    gidx_sb[:, o : o + g // 16],
                    g, regs[g], DIM,
                    single_packet=False,
                    queue_num=1 + qn % (NQ - 1),
                )

            # --- neg k2-7 as SWDGE static DMAs behind the gathers --------
            # (their ~2.9MB stays off the HWDGE queues that must deliver
            # the ctx tiles early; the DVE consumes them late)
            for k in range(2, K):
                m = M_NEG[k]
                nc.gpsimd.dma_start(
                    out=neg_t[k][:].rearrange("p (t d) -> p t d", d=DIM)[
                        :, : m // P, :
                    ],
                    in_=negA[noffs[k] : noffs[k] + m, :].rearrange(
                        "(p t) d -> p t d", p=P
                    ),
                )

            # --- ctx pooling on the PE: identity matmuls accumulate the 10
            # position tiles into PSUM f32, one 512-col quarter per bank.
            acc_ps = [
                psp.tile([P, QW], F32, tag=f"acc{q}", name=f"acc{q}")
                for q in range(BL // QW)
            ]
            # quarters 0-2 read only fp8 dense tiles (no gather deps); the
            # q3 chain (cols [1536, 2048)) mixes per-c dense/gather column
            # subranges — every column is first written by c0's start=True
            # full-width matmul, so later subrange matmuls all accumulate.
            NQ3 = (BL // QW) - 1
            for q in range(NQ3):
                for c in range(C):
                    rhs_tile = ctx_t[c] if c < CTX_N8 else ctxd_t[c]
                    nc.tensor.matmul(
                        out=acc_ps[q][:],
                        lhsT=ident8[:],
                        rhs=rhs_tile[:, q * QW : (q + 1) * QW],
                        start=(c == 0),
                        stop=(c == C - 1),
                    )
            q3_parts = []  # (rhs_ap, out_lo, out_hi, fp8)
            for c in range(C):
                if c < CTX_N8:
                    q3_parts.append((ctx_t[c][:, NQ3 * QW :], 0, QW, True))
                else:
                    mt = M_CTX[c] // P
                    dcols = (mt - 12) * DIM  # dense cols within q3
                    if dcols > 0:
                        q3_parts.append(
                            (ctxd_t[c][:, 12 * DIM :], 0, dcols, True)
                        )
                    q3_parts.append((ctxg_t[c][:], dcols, QW, False))
            for i, (rhs, lo, hi, fp8) in enumerate(q3_parts):
                nc.tensor.matmul(
                    out=acc_ps[NQ3][:, lo:hi],
                    lhsT=(ident8 if fp8 else ident)[:],
                    rhs=rhs,
                    start=(i == 0),
                    stop=(i == len(q3_parts) - 1),
                )

            # --- PSUM -> SBUF src_acc on the (idle) Scalar engine: DVE reads
            # PSUM at half rate, so the muls read an SBUF copy instead.
            src_acc = idxp.tile([P, BL], EMB, tag="srcacc", name="srcacc")
            for q in range(BL // QW):
                nc.scalar.activation(
                    out=src_acc[:, q * QW : (q + 1) * QW],
                    in_=acc_ps[q][:],
                    func=mybir.ActivationFunctionType.Copy,
                )

            # --- negatives: mul + fold + reduce on the DVE ----------------
            pred_halves = [
                epip.tile([P, KH * T], F32, tag="predlo", name="predlo"),
                epip.tile([P, KH * T], F32, tag="predhi", name="predhi"),
            ]
            # k0/k1 run in a 3/4 + 1/4 split: slots t<12 touch no gathered
            # data, so the DVE starts on them while the q3 (t>=12) gathers
            # and their PSUM chain are still in flight.  Remaining k's run
            # full-width (the per-op ~385ns DVE fixed cost makes finer
            # splits a net loss).
            TSPL = 12
            NSPLIT = 3  # how many leading k's run split
            prods, folds = {}, {}
            for k in range(4):
                prods[k] = prodp.tile([P, BL], EMB, tag="prodch", name=f"prod{k}")
                folds[k] = foldp.tile([P, BL // 2], EMB, tag="foldch", name=f"fold{k}")

            def _piece(k, lo, hi):
                kk = k % KH
                pv = prods[k][:].rearrange("p (t d) -> p t d", d=DIM)
                fv = folds[k][:].rearrange("p (t d) -> p t d", d=DIM // 2)
                nc.vector.tensor_mul(
                    out=prods[k][:, lo * DIM : hi * DIM],
                    in0=src_acc[:, lo * DIM : hi * DIM],
                    in1=neg_t[k][:, lo * DIM : hi * DIM],
                )
                nc.vector.tensor_add(
                    out=fv[:, lo:hi, :],
                    in0=pv[:, lo:hi, : DIM // 2],
                    in1=pv[:, lo:hi, DIM // 2 :],
                )
                nc.vector.tensor_reduce(
                    out=pred_halves[k // KH][:, kk * T + lo : kk * T + hi],
                    in_=fv[:, lo:hi, :],
                    axis=mybir.AxisListType.X,
                    op=mybir.AluOpType.add,
                )

            for k in range(NSPLIT):
                _piece(k, 0, TSPL)
            for k in range(NSPLIT):
                _piece(k, TSPL, T)
            _piece(3, 0, T)
            # k4-7: separate full-rate muls write adjacent halves of a
            # shared double-width prod tile, then ONE fold + ONE reduce
            # cover the pair (saves the ~385ns DVE fixed cost twice per
            # pair; no broadcast operands, no coupled input tiles).
            for j in range(2):
                ka, kb = 4 + 2 * j, 5 + 2 * j
                kk = ka % KH
                prod2 = prodp.tile(
                    [P, 2 * BL], EMB, tag="prodpair", name=f"prodp{j}"
                )
                fold2 = foldp.tile([P, BL], EMB, tag="foldpair", name=f"foldp{j}")
                for s, k in enumerate((ka, kb)):
                    nc.vector.tensor_mul(
                        out=prod2[:, s * BL : (s + 1) * BL],
                        in0=src_acc[:],
                        in1=neg_t[k][:],
                    )
                pv2 = prod2[:].rearrange("p (t d) -> p t d", d=DIM)
                fv2 = fold2[:].rearrange("p (t d) -> p t d", d=DIM // 2)
                nc.vector.tensor_add(
                    out=fv2, in0=pv2[:, :, : DIM // 2], in1=pv2[:, :, DIM // 2 :]
                )
                nc.vector.tensor_reduce(
                    out=pred_halves[1][:, kk * T : (kk + 2) * T],
                    in_=fv2,
                    axis=mybir.AxisListType.X,
                    op=mybir.AluOpType.add,
                )

            # --- epilogue: wm * (softplus(pred) - pred*label), sum over K -
            # softplus composed as relu(x) + ln(1 + exp(-|x|)) (no softplus
            # ACT table in this build).  Done per k-half so the first half
            # overlaps the second half's reduces.
            for hh in range(2):
                pred = pred_halves[hh]
                wm = wml_sb[:, hh * KH * T : (hh + 1) * KH * T]
                lab = wml_sb[:, (K + hh * KH) * T : (K + (hh + 1) * KH) * T]
                sp_a = epip.tile([P, KH * T], F32, tag=f"spa{hh}", name=f"spa{hh}")
                nc.scalar.activation(
                    out=sp_a[:], in_=pred[:], func=mybir.ActivationFunctionType.Abs
                )
                nc.scalar.activation(
                    out=sp_a[:], in_=sp_a[:],
                    func=mybir.ActivationFunctionType.Exp, scale=-1.0,
                )
                nc.scalar.activation(
                    out=sp_a[:], in_=sp_a[:],
                    func=mybir.ActivationFunctionType.Ln, bias=1.0,
                )
                sp_r = epip.tile([P, KH * T], F32, tag=f"spr{hh}", name=f"spr{hh}")
                nc.scalar.activation(
                    out=sp_r[:], in_=pred[:], func=mybir.ActivationFunctionType.Relu
                )
                t1 = epip.tile([P, KH * T], F32, tag=f"t1{hh}", name=f"t1{hh}")
                nc.vector.tensor_mul(out=t1[:], in0=pred[:], in1=lab)
                nc.vector.tensor_sub(out=sp_r[:], in0=sp_r[:], in1=t1[:])
                nc.vector.tensor_add(out=sp_r[:], in0=sp_r[:], in1=sp_a[:])
                nc.vector.tensor_mul(out=sp_r[:], in0=sp_r[:], in1=wm)
                nh = epip.tile([P, T], F32, tag=f"nh{hh}", name=f"nh{hh}")
                nc.vector.tensor_reduce(
                    out=nh[:],
                    in_=sp_r[:].rearrange("p (k t) -> p t k", k=KH),
                    axis=mybir.AxisListType.X,
                    op=mybir.AluOpType.add,
                )
                # each half's output DMA overlaps the other half's epilogue
                nc.sync.dma_start(out=out[:, hh * T : (hh + 1) * T], in_=nh[:])

    _split_multiwait(nc)
    lower_extended_insts(nc)

    # Hoist the library reload to the very front of the main block: the
    # ~10 us Q7 ucode load then overlaps the Bass preamble (sem init +
    # all-core start barrier) instead of serializing after it.  The reload
    # has no register or semaphore operands, and the const MEMSETs are
    # native Pool ops (not library ucode), so reordering is safe.
    mainb = nc.m.functions[0].blocks[0]
    il = mainb.instructions
    reloads = [i for i in il if "Reload" in type(i).__name__
               or getattr(i, "op_name", "") == "PseudoReloadLibraryIndex"]
    for r in reloads:
        il.remove(r)
    for pos, r in enumerate(reloads):
        il.insert(pos, r)
    _cached_nc = nc
    return nc


def _wrap(flat):
    """[n] int16 (flat[i] gathers to out slot (i%128, i//128)) -> the
    dma_gather idx tile layout: [16, n//16] with (p, s) = flat[s*16+p],
    replicated to 128 partitions."""
    n = flat.shape[0]
    return np.tile(flat.reshape(n // 16, 16).T, (8, 1))


def _prep_side(vals, emb, m_levels, tab_rows, dense_dt=TAB_DT):
    """Shared host prep for one side (ctx or neg) of one core.

    vals: [BL, J] int64 vocab ids.  Returns (table, denseA, idx_np, order)
    where order is the per-example column permutation applied (singles
    first), so callers can permute slot-aligned payloads identically.
    """
    J = vals.shape[1]
    u, inv = np.unique(vals.ravel(), return_inverse=True)
    ids = inv.reshape(BL, J)
    cnt = np.bincount(inv)
    singles = cnt[ids] == 1
    order = np.argsort(~singles, axis=1, kind="stable")
    sids = np.take_along_axis(ids, order, axis=1)
    tab = np.zeros((tab_rows, DIM), dtype=TAB_DT)
    tab[: len(u)] = emb[u].astype(TAB_DT)
    # dense streams packed p-major per level: row r <-> slot (p=r//mt, t=r%mt)
    # fully-dense levels go to the fp8 stream, mixed levels to the bf16 one.
    segs = []
    for j in range(J):
        m = m_levels[j]
        segs.append(
            tab[sids[:m, j].reshape(m // P, P).T.ravel()].astype(dense_dt)
        )
    denseA = np.concatenate(segs, axis=0)
    gsegs = [
        _wrap(sids[m_levels[j] :, j].astype(np.int16))
        for j in range(J)
        if m_levels[j] < BL
    ]
    idx_np = np.concatenate(gsegs, axis=1)
    return tab, denseA, idx_np, order


def kernel(contexts, focus_word, weight_mask, labels, ctx_emb, neg_emb):
    contexts = np.asarray(contexts)
    focus_word = np.asarray(focus_word)
    weight_mask = np.asarray(weight_mask, dtype=np.float32)
    labels = np.asarray(labels, dtype=np.float32)
    ctx_emb = np.asarray(ctx_emb, dtype=np.float32)
    neg_emb = np.asarray(neg_emb, dtype=np.float32)

    nc = _build()

    in_maps = []
    dens = []
    for i in range(NCORES):
        sl = slice(i * BL, (i + 1) * BL)
        ctx_i = contexts[sl].astype(np.int64)  # [BL, C]
        foc_i = focus_word[sl].astype(np.int64)  # [BL, K]
        wm_i = weight_mask[sl]  # [BL, K]
        lab_i = labels[sl]

        ctab, ctxA8_np, ctx_idx_np, _ = _prep_side(
            ctx_i, ctx_emb, M_CTX, CTX_ROWS, dense_dt=TAB8_DT
        )
        ntab, negA_np, neg_idx_np, n_order = _prep_side(
            foc_i, neg_emb, M_NEG, NEG_ROWS
        )
        # negative slots were permuted per-example: permute wm/labels too
        wm_s = np.take_along_axis(wm_i, n_order, axis=1)
        lab_s = np.take_along_axis(lab_i, n_order, axis=1)

        # wm/lab to [P, K*T]: (p, k*T+t) = value[e = t*128+p, k]
        wm_r = wm_s.reshape(T, P, K).transpose(1, 2, 0).reshape(P, K * T)
        lab_r = lab_s.reshape(T, P, K).transpose(1, 2, 0).reshape(P, K * T)
        wml_np = np.concatenate([wm_r, lab_r], axis=1)

        in_maps.append(
            {
                "ctx_tab": ctab,
                "neg_tab": ntab,
                "ctxA8": np.ascontiguousarray(ctxA8_np),
                "negA": np.ascontiguousarray(negA_np),
                "gidx": np.ascontiguousarray(
                    np.concatenate([ctx_idx_np, neg_idx_np], axis=1)
                ),
                "ident": np.eye(P, dtype=TAB_DT),
                "ident8": np.eye(P, dtype=TAB8_DT),
                "wml": np.ascontiguousarray(wml_np),
            }
        )
        dens.append(wm_i.sum(axis=1))  # [BL] row denominators

    res = run_bass_kernel_spmd(nc, in_maps, core_ids=list(range(NCORES)))

    total = 0.0
    for i in range(NCORES):
        o = res.results[i]["out"]  # [P, 2T]: two K-half numerators
        num = o[:, :T] + o[:, T:]
        num_e = num.T.reshape(BL)  # [BL] in example order
        total += float((num_e.astype(np.float64) / dens[i].astype(np.float64)).sum())
    return np.float32(total / B)
